# revision 1
# baseline (speedup 1.0000x reference)
"""AstroEconomicTransformer on 8 Trainium2 NeuronCores.

Sharding: 8-way sequence-parallel over the B*S = 2048 tokens (256 tokens
per core; cores 0-3 hold batch 0, cores 4-7 batch 1). Activations live
feature-major on chip (x^T: features on partitions, tokens on the free
dim), so every linear layer is a W^T-stationary matmul with the token
dim streaming. Attention needs the full sequence of K/V per batch
element: each layer runs ONE fused bf16 AllGather of [K|V] within each
group of 4 cores, then bulk-loads the gathered K/V into SBUF; everything
else (LayerNorm, FFN, projections, softmax) is token-local.

Attention per head-pair: scores^T = k^T.T @ q^T (keys on partitions,
queries free), exp on the scalar engine with the 1/8 scale folded into
the q eviction and pbias as the activation bias (no max subtraction:
scores are O(1) by construction). Token-major V per pair is laid out
[h0 64 | h1 64 | one_h0 | one_h1] so two 64-wide matmuls produce both
heads' unnormalized ctx stacked across the partition halves of one psum
tile, and two 1-row matmuls accumulate the softmax denominators.
Normalization broadcasts the reciprocals with a K=1 matmul over a
(1, 2T) plane. LayerNorm's rsqrt runs as exp(-0.5*ln(var+eps)) so the
Act engine never leaves the exp/ln table. The V bias is folded on host
into the attention output bias (bo_eff = bo + Wo @ bv, exact because
softmax rows sum to one).
"""

import numpy as np

B, S = 2, 1024
D, H, L, DFF = 1024, 16, 6, 4096
NM, NA, OUT = 10, 20, 1
HD = D // H
EPS = 1e-5

NCORES = 8
GPC = 4  # cores per batch group
T = (B * S) // NCORES  # 256 tokens per core
GROUPS = [[0, 1, 2, 3], [4, 5, 6, 7]]
P = 128
DT = D // P  # 8 feature tiles
FT = DFF // P  # 32 dff tiles
TT = T // P  # 2 token tiles per core
ST = S // P  # 8 key tiles per sequence
NPAIR = H // 2

PV = 2 * HD + 2  # 130: per-pair v row [h0 64 | h1 64 | one one]
VROW = NPAIR * PV  # 1040
KELEM = D * T  # 262144
VELEM = T * VROW  # 266240
KVELEM = KELEM + VELEM

_RUNNER = None
REPS = 1


class _Cols:
    """Allocates columns in the (128, n) bias/constant matrix."""

    def __init__(self):
        self.cols = []

    def add(self, mat):  # mat: (128, n) -> first col index
        i = len(self.cols)
        self.cols.extend(np.asarray(mat, np.float32).T)
        return i

    def array(self):
        return np.stack(self.cols, axis=1).astype(np.float32)


def _group_kxm(wT, km, mm_):
    """(N, K, M) pre-transposed weight -> (N*mm_, P, km*P) SBUF tile images."""
    n = wT.shape[0]
    g = wT.reshape(n, km, P, mm_, P).transpose(0, 3, 2, 1, 4)
    return np.ascontiguousarray(g.reshape(n * mm_, P, km * P))


def _prep_host(inputs):
    f32 = np.float32
    g = {k: np.asarray(v, f32) for k, v in inputs.items()}

    cols = _Cols()
    idx = {}
    bemb = np.concatenate([g["bm"], g["ba"]])
    idx["bemb"] = cols.add(bemb.reshape(DT, P).T)
    for l in range(L):
        idx[f"bq{l}"] = cols.add((g["bq"][l] * 0.125).reshape(DT, P).T)
        idx[f"bk{l}"] = cols.add(g["bk"][l].reshape(DT, P).T)
        bo_eff = g["bo"][l] + g["Wo"][l] @ g["bv"][l]
        idx[f"bo{l}"] = cols.add(bo_eff.reshape(DT, P).T)
        idx[f"b1{l}"] = cols.add(g["b1"][l].reshape(FT, P).T)
        idx[f"b2{l}"] = cols.add(g["b2"][l].reshape(DT, P).T)
        idx[f"g1{l}"] = cols.add(g["ln1_g"][l].reshape(DT, P).T)
        idx[f"be1{l}"] = cols.add(g["ln1_b"][l].reshape(DT, P).T)
        idx[f"g2{l}"] = cols.add(g["ln2_g"][l].reshape(DT, P).T)
        idx[f"be2{l}"] = cols.add(g["ln2_b"][l].reshape(DT, P).T)
        idx[f"pb{l}"] = cols.add(np.tile(g["pbias"][l][None, :], (P, 1)))
    idx["gf"] = cols.add(g["lnf_g"].reshape(DT, P).T)
    idx["bef"] = cols.add(g["lnf_b"].reshape(DT, P).T)
    idx["bout"] = cols.add(np.full((P, 1), g["bout"][0], f32))
    idx["eps"] = cols.add(np.full((P, 1), EPS, f32))
    idx["zero"] = cols.add(np.zeros((P, 1), f32))
    bcols = cols.array()
    idx["_nbc"] = bcols.shape[1]

    import ml_dtypes

    bf = lambda a: np.ascontiguousarray(a).astype(ml_dtypes.bfloat16)
    tr = lambda w: w.transpose(0, 2, 1)
    # Wv reordered so v rows are [pair][h0 64 | h1 64 | pad pad]:
    # output channel order within each 512-slab stays grouped by n-group
    # of 8 heads; we emit psum (128, 512) = heads 8n..8n+7 and copy into
    # strided pair slots, so WvT stays as-is.
    shared = {
        "bcols": bcols,
        "WmT": np.ascontiguousarray(g["Wm"].T),
        "WaT": np.ascontiguousarray(g["Wa"].T),
        "Wq_g": bf(_group_kxm(tr(g["Wq"]), DT, DT)),
        "Wk_g": bf(_group_kxm(tr(g["Wk"]), DT, DT)),
        "WvT": bf(tr(g["Wv"])),  # rhs-moving, row slabs
        "Wo_g": bf(_group_kxm(tr(g["Wo"]), DT, DT)),
        "W1_g": bf(_group_kxm(tr(g["W1"]), DT, FT)),
        "W2_g": bf(_group_kxm(tr(g["W2"]), FT, DT)),
        "WoutT": np.ascontiguousarray(g["Wout"].T),
        "onesb": np.ones((P, P), f32),
    }

    per_core = []
    peT_full = np.ascontiguousarray(g["pe"][0].T)
    for c in range(NCORES):
        b, chunk = c // GPC, c % GPC
        r0 = chunk * T
        per_core.append(
            {
                "mktT": np.ascontiguousarray(g["market_data"][b, r0 : r0 + T, :].T),
                "astT": np.ascontiguousarray(g["astro_data"][b, r0 : r0 + T, :].T),
                "peT": np.ascontiguousarray(peT_full[:, r0 : r0 + T]),
            }
        )
    return shared, per_core, idx


# ---------------------------------------------------------------- device kernel
def _build(idx):
    from contextlib import ExitStack

    import concourse.mybir as mybir
    import concourse.tile as tile
    from concourse import bacc

    dt = mybir.dt
    F32, F32R, BF16 = dt.float32, dt.float32r, dt.bfloat16
    AF = mybir.ActivationFunctionType
    ALU = mybir.AluOpType

    nc = bacc.Bacc("TRN2", debug=False, num_devices=NCORES)

    NBC = idx["_nbc"]

    mktT = nc.declare_dram_parameter("mktT", [NM, T], F32R, isOutput=False)
    astT = nc.declare_dram_parameter("astT", [NA, T], F32R, isOutput=False)
    peT = nc.declare_dram_parameter("peT", [D, T], F32, isOutput=False)
    bcols_d = nc.declare_dram_parameter("bcols", [P, NBC], F32, isOutput=False)
    WmT = nc.declare_dram_parameter("WmT", [NM, D // 2], F32R, isOutput=False)
    WaT = nc.declare_dram_parameter("WaT", [NA, D // 2], F32R, isOutput=False)
    Wq_g = nc.declare_dram_parameter("Wq_g", [L * DT, P, DT * P], BF16, isOutput=False)
    Wk_g = nc.declare_dram_parameter("Wk_g", [L * DT, P, DT * P], BF16, isOutput=False)
    WvT = nc.declare_dram_parameter("WvT", [L, D, D], BF16, isOutput=False)
    Wo_g = nc.declare_dram_parameter("Wo_g", [L * DT, P, DT * P], BF16, isOutput=False)
    W1_g = nc.declare_dram_parameter("W1_g", [L * FT, P, DT * P], BF16, isOutput=False)
    W2_g = nc.declare_dram_parameter("W2_g", [L * DT, P, FT * P], BF16, isOutput=False)
    WoutT = nc.declare_dram_parameter("WoutT", [D, OUT], F32R, isOutput=False)
    ones_d = nc.declare_dram_parameter("onesb", [P, P], F32R, isOutput=False)
    y_out = nc.declare_dram_parameter("y", [1, T], F32, isOutput=True)

    kv_in = [nc.dram_tensor(f"kv_in{l}", [KVELEM], BF16) for l in range(L)]
    kv_ag = [nc.dram_tensor(f"kv_ag{l}", [GPC, KVELEM], BF16) for l in range(L)]

    with tile.TileContext(nc) as tc, ExitStack() as ctx:
        def pool(name, bufs, space="SBUF"):
            return ctx.enter_context(tc.tile_pool(name=name, bufs=bufs, space=space))

        singles = pool("singles", 1)
        xp = pool("xarr", 3)
        xbp = pool("xbf", 2)
        qp = pool("qarr", 1)
        cxp = pool("ctxarr", 1)
        hp = pool("harr", 1)
        wp = pool("wrow", 12)  # qkvo/W1 k-groups + WvT row slabs (bf16)
        w2p = pool("w2grp", 4)  # (128,4096) bf16
        kvp = pool("kvloc", 2)
        kvap = pool("kvall", 1)
        exp_p = pool("exparr", 5)
        sqp = pool("sqp", 2)
        bcp = pool("bcp", 1)
        lntp = pool("lntp", 4)
        sp = pool("small", 3)
        embp = pool("embp", 2)
        nrmp = pool("nrmp", 2)

        ps = pool("ps", 5, space="PSUM")
        psx = pool("psx", 3, space="PSUM")

        bc = singles.tile([P, NBC], F32)
        nc.sync.dma_start(bc[:], bcols_d[:])
        onesb = singles.tile([P, P], F32R)
        nc.sync.dma_start(onesb[:], ones_d[:])
        pe_sb = singles.tile([P, DT * T], F32)
        nc.sync.dma_start(pe_sb[:].rearrange("p (a t) -> p a t", t=T), peT[:].rearrange("(a p) t -> p a t", p=P))
        wout_sb = singles.tile([P, DT], F32R)
        nc.sync.dma_start(wout_sb[:].rearrange("p (a o) -> p a o", o=OUT), WoutT[:].rearrange("(a p) o -> p a o", p=P))

        def col(name, j=0, rows=P):
            return bc[0:rows, idx[name] + j : idx[name] + j + 1]

        def mm(out, lhsT, rhs, start, stop):
            nc.tensor.matmul(out, lhsT, rhs, start=start, stop=stop)

        for _rep in range(REPS):
            # ======================================================== embed
            x = xp.tile([P, DT * T], F32R, tag="xarr")
            xb = xbp.tile([P, DT * T], BF16, tag="xbf", name="xb_emb")
            in_sb = embp.tile([NA, 2 * T], F32R, tag="embin")
            nc.sync.dma_start(in_sb[0:NM, 0:T], mktT[:])
            nc.sync.dma_start(in_sb[0:NA, T : 2 * T], astT[:])
            wemb = embp.tile([NA, D // 2], F32R, tag="wemb")
            nc.sync.dma_start(wemb[0:NM, :], WmT[:])
            wemb2 = embp.tile([NA, D // 2], F32R, tag="wemb")
            nc.sync.dma_start(wemb2[:], WaT[:])
            for m in range(DT):
                pm = ps.tile([P, T], F32, tag="ps")
                if m < 4:
                    w, nin, toff, mo = wemb, NM, 0, m
                else:
                    w, nin, toff, mo = wemb2, NA, T, m - 4
                mm(pm[:], w[0:nin, mo * P : (mo + 1) * P], in_sb[0:nin, toff : toff + T],
                   start=True, stop=True)
                nc.vector.scalar_tensor_tensor(
                    x[:, m * T : (m + 1) * T], pm[:], col("bemb", m),
                    pe_sb[:, m * T : (m + 1) * T], ALU.add, ALU.add,
                )
                nc.gpsimd.tensor_scalar_mul(
                    xb[:, m * T : (m + 1) * T], x[:, m * T : (m + 1) * T], 1.0
                )

            # ============================================================ helpers
            def load_group(pl, src_row, ntiles, tag):
                t = pl.tile([P, ntiles * P], BF16, tag=tag)
                nc.sync.dma_start(t[:], src_row)
                return t

            def proj_fm(wg_d, row0, src, bias_fn, dst_fn, nm=DT, nk=DT):
                """Feature-major projection: dst[m] = act(W @ src + bias)."""
                for m in range(nm):
                    grp = load_group(w2p if nk == FT else wp, wg_d[row0 + m, :, :], nk,
                                     "w2grp" if nk == FT else "wrow")
                    pr = ps.tile([P, T], F32, tag="ps")
                    for kk in range(nk):
                        mm(pr[:], grp[:, kk * P : (kk + 1) * P],
                           src(kk), start=(kk == 0), stop=(kk == nk - 1))
                    dst_fn(m, pr, bias_fn(m))

            def layernorm(src_tiles, gname, bname, dst, dst_bf=None):
                """dst = LN(src) with gamma/beta; Act engine untouched."""
                s_ps = psx.tile([1, T], F32, tag="aux")
                s2_ps = psx.tile([1, T], F32, tag="aux")
                for m in range(DT):
                    eng = nc.gpsimd if m % 2 else nc.vector
                    sq = sqp.tile([P, T], F32R, tag="sq")
                    eng.tensor_mul(sq[:], src_tiles[m], src_tiles[m])
                    mm(s2_ps[:], onesb[:, 0:1], sq[:], start=(m == 0), stop=(m == DT - 1))
                for m in range(DT):
                    mm(s_ps[:], onesb[:, 0:1], src_tiles[m], start=(m == 0), stop=(m == DT - 1))
                mu = sp.tile([1, T], F32R, tag="stat1")
                nc.vector.tensor_scalar_mul(mu[:], s_ps[:], 1.0 / D)
                ex2 = sp.tile([1, T], F32, tag="stat1")
                nc.vector.tensor_scalar_mul(ex2[:], s2_ps[:], 1.0 / D)
                mu2 = sp.tile([1, T], F32, tag="stat1")
                nc.vector.tensor_mul(mu2[:], mu[:], mu[:])
                var = sp.tile([1, T], F32, tag="stat1")
                nc.vector.scalar_tensor_tensor(var[:], mu2[:], -1.0, ex2[:],
                                               ALU.mult, ALU.add)
                # rs = rsqrt(var+eps) = exp(-0.5*ln(var+eps)): stays in the
                # exp/ln act table — no act-func-set reloads anywhere.
                lv = sp.tile([1, T], F32, tag="stat1")
                nc.scalar.activation(lv[:], var[:], AF.Ln,
                                     bias=col("eps", rows=1), scale=1.0)
                rs = sp.tile([1, T], F32R, tag="stat1")
                with nc.allow_low_precision(reason="fp32r feeds the broadcast matmul"):
                    nc.scalar.activation(rs[:], lv[:], AF.Exp,
                                         bias=col("zero", rows=1), scale=-0.5)
                mub_ps = psx.tile([P, T], F32, tag="aux")
                mm(mub_ps[:], onesb[0:1, :], mu[:], start=True, stop=True)
                rsb_ps = psx.tile([P, T], F32, tag="aux")
                mm(rsb_ps[:], onesb[0:1, :], rs[:], start=True, stop=True)
                rsb = bcp.tile([P, T], F32, tag="bcast")
                nc.scalar.copy(rsb[:], rsb_ps[:])
                for m in range(DT):
                    eng = nc.vector if m % 2 else nc.gpsimd
                    eng2 = nc.gpsimd if m % 2 else nc.vector
                    t1 = lntp.tile([P, T], F32, tag="lnt")
                    nc.vector.tensor_sub(t1[:], src_tiles[m], mub_ps[:])
                    t2 = lntp.tile([P, T], F32, tag="lnt")
                    eng.tensor_mul(t2[:], t1[:], rsb[:])
                    eng.tensor_scalar(
                        dst[:, m * T : (m + 1) * T], t2[:], col(gname, m), col(bname, m),
                        ALU.mult, ALU.add,
                    )
                    if dst_bf is not None:
                        eng2.tensor_scalar(
                            dst_bf[:, m * T : (m + 1) * T], t2[:],
                            col(gname, m), col(bname, m), ALU.mult, ALU.add,
                        )

            # ============================================================ layers
            for l in range(L):
                # ---- k projection (feature-major) -> kv_in[0:KELEM] bf16
                def k_dst(m, pr, bias, l=l):
                    kt = kvp.tile([P, T], BF16, tag="kloc")
                    nc.vector.tensor_scalar_add(kt[:], pr[:], bias)
                    nc.sync.dma_start(
                        kv_in[l][0:KELEM].rearrange("(r t) -> r t", t=T)[m * P : (m + 1) * P, :],
                        kt[:],
                    )

                proj_fm(Wk_g, l * DT, lambda kk, xb=xb: xb[:, kk * T : (kk + 1) * T],
                        lambda m, l=l: col(f"bk{l}", m), k_dst)

                # ---- v projection (token-major, pair layout + ones)
                vls = []
                for mt in range(TT):
                    vl = kvp.tile([P, VROW], BF16, tag="vloc")
                    # ones columns at [pair*130 + 128, pair*130 + 129]
                    nc.vector.memset(
                        vl[:].rearrange("p (pr c) -> p pr c", c=PV)[:, :, 2 * HD : PV],
                        1.0,
                    )
                    vls.append(vl)
                for n in range(2):
                    pvs = [ps.tile([P, 512], F32, tag="ps", name=f"pv{l}_{n}_{mt}")
                           for mt in range(TT)]
                    for kk in range(DT):
                        slab = wp.tile([P, 512], BF16, tag="wrow")
                        nc.sync.dma_start(
                            slab[:], WvT[l, kk * P : (kk + 1) * P, n * 512 : (n + 1) * 512]
                        )
                        for mt in range(TT):
                            mm(pvs[mt][:], xb[:, kk * T + mt * P : kk * T + (mt + 1) * P],
                               slab[:], start=(kk == 0), stop=(kk == DT - 1))
                    for mt in range(TT):
                        # psum heads 8n..8n+7 -> pairs 4n..4n+3, head j slot
                        vl4 = vls[mt][:].rearrange("p (pr c) -> p pr c", c=PV)[
                            :, 4 * n : 4 * (n + 1), 0 : 2 * HD
                        ].rearrange("p pr (j h) -> p pr j h", h=HD)
                        nc.scalar.copy(
                            vl4,
                            pvs[mt][:].rearrange("p (pr j h) -> p pr j h", h=HD, j=2),
                        )
                for mt in range(TT):
                    nc.sync.dma_start(
                        kv_in[l][KELEM:].rearrange("(r t) -> r t", t=VROW)[mt * P : (mt + 1) * P, :],
                        vls[mt][:],
                    )
                nc.gpsimd.collective_compute(
                    "AllGather", ALU.bypass, replica_groups=GROUPS,
                    ins=[kv_in[l][:].opt()], outs=[kv_ag[l][:].opt()],
                )

                # ---- q projection (feature-major, pre-scaled by 1/8); overlaps AG
                q = qp.tile([P, DT * T], BF16, tag="qarr")

                def q_dst(m, pr, bias, q=q):
                    nc.vector.tensor_scalar(
                        q[:, m * T : (m + 1) * T], pr[:], 0.125, bias, ALU.mult, ALU.add
                    )

                proj_fm(Wq_g, l * DT, lambda kk, xb=xb: xb[:, kk * T : (kk + 1) * T],
                        lambda m, l=l: col(f"bq{l}", m), q_dst)

                # ---- bulk-load gathered K/V into SBUF (HWDGE, per-chunk)
                k_all = kvap.tile([P, GPC * DT * T], BF16, tag="kall", name=f"kall{l}")
                v_all = kvap.tile([P, GPC * TT * VROW], BF16, tag="vall", name=f"vall{l}")
                for c in range(GPC):
                    nc.scalar.dma_start(
                        k_all[:, c * DT * T : (c + 1) * DT * T].rearrange(
                            "p (a t) -> p a t", t=T),
                        kv_ag[l][c, 0:KELEM].rearrange("(a p t) -> p a t", p=P, t=T),
                    )
                for c in range(GPC):
                    nc.gpsimd.dma_start(
                        v_all[:, c * TT * VROW : (c + 1) * TT * VROW].rearrange(
                            "p (a v) -> p a v", v=VROW),
                        kv_ag[l][c, KELEM:].rearrange("(a p v) -> p a v", p=P, v=VROW),
                    )

                # ---- attention, one head pair per q partition tile
                ctxa = cxp.tile([P, DT * T], BF16, tag="ctxarr")
                for p in range(NPAIR):
                    ea = exp_p.tile([P, ST * T], BF16, tag="exparr")
                    eb = exp_p.tile([P, ST * T], BF16, tag="exparr")
                    for c in range(GPC):
                        for h01 in range(2):
                            o = h01 * HD
                            pscore = ps.tile([P, 2 * T], F32, tag="ps")
                            for sub in range(TT):
                                kcol = c * DT * T + p * T + sub * P
                                mm(pscore[:, sub * T : (sub + 1) * T],
                                   k_all[o : o + HD, kcol : kcol + P],
                                   q[o : o + HD, p * T : (p + 1) * T], start=True, stop=True)
                            dst = ea if h01 == 0 else eb
                            nc.scalar.activation(
                                dst[:, TT * c * T : TT * (c + 1) * T], pscore[:], AF.Exp,
                                bias=col(f"pb{l}", 2 * p + h01), scale=1.0,
                            )
                    pctx = psx.tile([P, T], F32, tag="aux")
                    pden0 = psx.tile([1, T], F32, tag="aux")
                    pden1 = psx.tile([1, T], F32, tag="aux")
                    for tk in range(ST):
                        c, sub = tk // TT, tk % TT
                        vcol = c * TT * VROW + sub * VROW + p * PV
                        eacol = ea[:, tk * T : (tk + 1) * T]
                        ebcol = eb[:, tk * T : (tk + 1) * T]
                        # both heads' ctx stacked across partition halves
                        mm(pctx[0:HD, :], v_all[:, vcol : vcol + HD], eacol,
                           start=(tk == 0), stop=(tk == ST - 1))
                        nc.tensor.matmul(pctx[HD : 2 * HD, :],
                                         v_all[:, vcol + HD : vcol + 2 * HD], ebcol,
                                         start=(tk == 0), stop=(tk == ST - 1),
                                         tile_position=(0, HD))
                        mm(pden0[:], v_all[:, vcol + 2 * HD : vcol + 2 * HD + 1], eacol,
                           start=(tk == 0), stop=(tk == ST - 1))
                        mm(pden1[:], v_all[:, vcol + 2 * HD + 1 : vcol + 2 * HD + 2], ebcol,
                           start=(tk == 0), stop=(tk == ST - 1))
                    rec = nrmp.tile([1, 2 * T], F32R, tag="rec")
                    with nc.allow_low_precision(reason="fp32r feeds the broadcast matmul"):
                        nc.vector.reciprocal(rec[0:1, 0:T], pden0[:])
                        nc.vector.reciprocal(rec[0:1, T : 2 * T], pden1[:])
                    pbc = ps.tile([P, 2 * T], F32, tag="ps")
                    mm(pbc[:], onesb[0:1, :], rec[0:1, :], start=True, stop=True)
                    bcsb = nrmp.tile([P, 2 * T], F32, tag="bc128")
                    nc.scalar.copy(bcsb[:], pbc[:])
                    nc.vector.tensor_mul(ctxa[0:HD, p * T : (p + 1) * T],
                                         pctx[0:HD, :], bcsb[0:HD, 0:T])
                    nc.vector.tensor_mul(ctxa[HD : 2 * HD, p * T : (p + 1) * T],
                                         pctx[HD : 2 * HD, :], bcsb[HD : 2 * HD, T : 2 * T])

                # ---- out projection + residual + LN1
                x1p = xp.tile([P, DT * T], F32R, tag="xarr")

                def o_dst(m, pr, bias, x1p=x1p, x=x):
                    eng = nc.vector
                    eng.scalar_tensor_tensor(
                        x1p[:, m * T : (m + 1) * T], pr[:], bias,
                        x[:, m * T : (m + 1) * T], ALU.add, ALU.add,
                    )

                proj_fm(Wo_g, l * DT, lambda kk: ctxa[:, kk * T : (kk + 1) * T],
                        lambda m, l=l: col(f"bo{l}", m), o_dst)
                x1 = xp.tile([P, DT * T], F32R, tag="xarr")
                x1b = xbp.tile([P, DT * T], BF16, tag="xbf", name=f"x1b_{l}")
                layernorm([x1p[:, m * T : (m + 1) * T] for m in range(DT)],
                          f"g1{l}", f"be1{l}", x1, dst_bf=x1b)

                # ---- FFN
                harr = hp.tile([P, FT * T], BF16, tag="harr")

                def h_dst(mf, pr, bias, harr=harr):
                    eng = nc.vector
                    eng.tensor_scalar(
                        harr[:, mf * T : (mf + 1) * T], pr[:], bias, 0.0, ALU.add, ALU.max
                    )

                proj_fm(W1_g, l * FT, lambda kk, x1b=x1b: x1b[:, kk * T : (kk + 1) * T],
                        lambda mf, l=l: col(f"b1{l}", mf), h_dst, nm=FT)

                x2p = xp.tile([P, DT * T], F32R, tag="xarr")

                def y_dst(m, pr, bias, x2p=x2p, x1=x1):
                    eng = nc.vector
                    eng.scalar_tensor_tensor(
                        x2p[:, m * T : (m + 1) * T], pr[:], bias,
                        x1[:, m * T : (m + 1) * T], ALU.add, ALU.add,
                    )

                proj_fm(W2_g, l * DT, lambda kf: harr[:, kf * T : (kf + 1) * T],
                        lambda m, l=l: col(f"b2{l}", m), y_dst, nk=FT)
                x2 = xp.tile([P, DT * T], F32R, tag="xarr")
                if l < L - 1:
                    xb = xbp.tile([P, DT * T], BF16, tag="xbf", name=f"xb_{l + 1}")
                else:
                    xb = None
                layernorm([x2p[:, m * T : (m + 1) * T] for m in range(DT)],
                          f"g2{l}", f"be2{l}", x2, dst_bf=xb)
                x = x2

            # ============================================================ head
            xf = xp.tile([P, DT * T], F32R, tag="xarr")
            layernorm([x[:, m * T : (m + 1) * T] for m in range(DT)], "gf", "bef", xf)
            pyf = psx.tile([1, T], F32, tag="aux")
            for m in range(DT):
                mm(pyf[:], wout_sb[:, m : m + 1], xf[:, m * T : (m + 1) * T],
                   start=(m == 0), stop=(m == DT - 1))
            ysb = sp.tile([1, T], F32, tag="stat1")
            nc.vector.tensor_scalar_add(ysb[:], pyf[:], col("bout", 0, rows=1))
            nc.sync.dma_start(y_out[:], ysb[:])

    nc.compile()
    return nc


# ---------------------------------------------------------------- runner
_SHARED_NAMES = frozenset(
    ["bcols", "WmT", "WaT", "Wq_g", "Wk_g", "WvT", "Wo_g", "W1_g", "W2_g",
     "WoutT", "onesb"]
)


def _make_runner(nc):
    import jax
    import concourse.mybir as mybir
    from concourse import bass2jax
    from jax.sharding import Mesh, PartitionSpec
    from jax.experimental.shard_map import shard_map

    bass2jax.install_neuronx_cc_hook()

    partition_name = nc.partition_id_tensor.name if nc.partition_id_tensor else None
    in_names, out_names, out_avals = [], [], []
    for alloc in nc.m.functions[0].allocations:
        if not isinstance(alloc, mybir.MemoryLocationSet):
            continue
        name = alloc.memorylocations[0].name
        if alloc.kind == "ExternalInput":
            if name != partition_name:
                in_names.append(name)
        elif alloc.kind == "ExternalOutput":
            out_names.append(name)
            out_avals.append(
                jax.core.ShapedArray(tuple(alloc.tensor_shape), mybir.dt.np(alloc.dtype))
            )
    n_params = len(in_names)
    n_outs = len(out_avals)
    all_in = in_names + out_names + ([partition_name] if partition_name else [])
    donate = tuple(range(n_params, n_params + n_outs))

    def _body(*args):
        operands = list(args)
        if partition_name is not None:
            operands.append(bass2jax.partition_id_tensor())
        return tuple(
            bass2jax._bass_exec_p.bind(
                *operands,
                out_avals=tuple(out_avals),
                in_names=tuple(all_in),
                out_names=tuple(out_names),
                lowering_input_output_aliases=(),
                sim_require_finite=True,
                sim_require_nnan=True,
                nc=nc,
            )
        )

    from jax.sharding import NamedSharding

    devices = jax.devices()[:NCORES]
    mesh = Mesh(np.asarray(devices), ("core",))
    repl_sharding = NamedSharding(mesh, PartitionSpec(None))
    core_sharding = NamedSharding(mesh, PartitionSpec("core"))
    in_specs = tuple(
        PartitionSpec(None) if name in _SHARED_NAMES else PartitionSpec("core")
        for name in in_names
    ) + (PartitionSpec("core"),) * n_outs
    out_specs = (PartitionSpec("core"),) * n_outs
    sharded = jax.jit(
        shard_map(_body, mesh=mesh, in_specs=in_specs, out_specs=out_specs,
                  check_rep=False),
        donate_argnums=donate,
        keep_unused=True,
    )

    class Runner:
        def upload(self, shared, per_core):
            ins = []
            for name in in_names:
                if name in _SHARED_NAMES:
                    ins.append((np.asarray(shared[name]), repl_sharding))
                else:
                    ins.append(
                        (
                            np.concatenate(
                                [np.asarray(per_core[c][name])
                                 for c in range(NCORES)],
                                axis=0,
                            ),
                            core_sharding,
                        )
                    )
            self.in_dev = [jax.device_put(a, s) for a, s in ins]
            self.zero_np = [
                np.zeros((NCORES * av.shape[0], *av.shape[1:]), av.dtype)
                for av in out_avals
            ]
            jax.block_until_ready(self.in_dev)

        def dispatch(self):
            zeros = [
                jax.device_put(z, core_sharding) for z in self.zero_np
            ]
            return sharded(*self.in_dev, *zeros)

        def collect(self, out_arrs):
            return [
                {
                    name: np.asarray(out_arrs[i]).reshape(NCORES, *out_avals[i].shape)[c]
                    for i, name in enumerate(out_names)
                }
                for c in range(NCORES)
            ]

        def run(self):
            import jax as _jax

            out_arrs = self.dispatch()
            _jax.block_until_ready(out_arrs)
            return self.collect(out_arrs)

    return Runner()


def get_runner(inputs):
    """Build (once) and return the runner with inputs uploaded."""
    global _RUNNER
    shared, per_core, idx = _prep_host(inputs)
    if _RUNNER is None:
        nc = _build(idx)
        _RUNNER = _make_runner(nc)
    _RUNNER.upload(shared, per_core)
    return _RUNNER


def kernel(**inputs) -> np.ndarray:
    runner = get_runner(inputs)
    res = runner.run()
    out = np.zeros((B, S, OUT), np.float32)
    for c in range(NCORES):
        b, chunk = c // GPC, c % GPC
        out[b, chunk * T : (chunk + 1) * T, 0] = res[c]["y"][0]
    return out



# revision 12
# speedup vs baseline: 2.6824x; 2.6824x over previous
"""AstroEconomicTransformer on 8 Trainium2 NeuronCores.

Sharding: 8-way sequence-parallel over the B*S = 2048 tokens (256 tokens
per core; cores 0-3 hold batch 0, cores 4-7 batch 1). Activations live
feature-major on chip (x^T: features on partitions, tokens on the free
dim), so every linear layer is a W^T-stationary matmul with the token
dim streaming. Attention needs the full sequence of K/V per batch
element: each layer runs TWO bf16 AllGathers (K right after the K
projection, V after the V projection) within each group of 4 cores, so
the V projection, Q projection and the first score matmuls overlap the
collectives; everything else (LayerNorm, FFN, projections, softmax) is
token-local.

Attention per head-pair: scores^T = k^T.T @ q^T (keys on partitions,
queries free), exp on the scalar engine with the 1/8 scale folded into
the q eviction and pbias as the activation bias (no max subtraction:
scores are O(1) by construction). Token-major V is laid out per head as
[v 64 | one] so the 65-column ctx matmul emits the softmax denominator
at psum partition 64 of the same accumulation — no separate denominator
matmuls. Both heads of a pair accumulate into one psum bank (h0 in
columns 0:256, h1 in 256:512), so one (1,512) reciprocal covers both
denominators; normalization broadcasts the reciprocals with a K=1
matmul. LayerNorm's rsqrt runs as exp(-0.5*ln(var+eps)) so the Act
engine never leaves the exp/ln table. The V bias is folded on host into
the attention output bias (bo_eff = bo + Wo @ bv, exact because softmax
rows sum to one).
"""

import numpy as np

B, S = 2, 1024
D, H, L, DFF = 1024, 16, 6, 4096
NM, NA, OUT = 10, 20, 1
HD = D // H
EPS = 1e-5

NCORES = 8
GPC = 4  # cores per batch group
T = (B * S) // NCORES  # 256 tokens per core
GROUPS = [[0, 1, 2, 3], [4, 5, 6, 7]]
P = 128
DT = D // P  # 8 feature tiles
FT = DFF // P  # 32 dff tiles
TT = T // P  # 2 token tiles per core
ST = S // P  # 8 key tiles per sequence
NPAIR = H // 2

PV = HD + 1  # 65: per-head v row [v 64 | one]
VROW = H * PV  # 1040
KELEM = D * T  # 262144
VELEM = T * VROW  # 266240

_RUNNER = None
REPS = 1


class _Cols:
    """Allocates columns in the (128, n) bias/constant matrix."""

    def __init__(self):
        self.cols = []

    def add(self, mat):  # mat: (128, n) -> first col index
        i = len(self.cols)
        self.cols.extend(np.asarray(mat, np.float32).T)
        return i

    def array(self):
        return np.stack(self.cols, axis=1).astype(np.float32)


def _group_kxm(wT, km, mm_):
    """(N, K, M) pre-transposed weight -> (N*mm_, P, km*P) SBUF tile images."""
    n = wT.shape[0]
    g = wT.reshape(n, km, P, mm_, P).transpose(0, 3, 2, 1, 4)
    return np.ascontiguousarray(g.reshape(n * mm_, P, km * P))


def _prep_host(inputs):
    f32 = np.float32
    g = {k: np.asarray(v, f32) for k, v in inputs.items()}

    cols = _Cols()
    idx = {}
    bemb = np.concatenate([g["bm"], g["ba"]])
    idx["bemb"] = cols.add(bemb.reshape(DT, P).T)
    for l in range(L):
        idx[f"bq{l}"] = cols.add((g["bq"][l] * 0.125).reshape(DT, P).T)
        idx[f"bk{l}"] = cols.add(g["bk"][l].reshape(DT, P).T)
        bo_eff = g["bo"][l] + g["Wo"][l] @ g["bv"][l]
        idx[f"bo{l}"] = cols.add(bo_eff.reshape(DT, P).T)
        idx[f"b1{l}"] = cols.add(g["b1"][l].reshape(FT, P).T)
        idx[f"b2{l}"] = cols.add(g["b2"][l].reshape(DT, P).T)
        idx[f"g1{l}"] = cols.add(g["ln1_g"][l].reshape(DT, P).T)
        idx[f"be1{l}"] = cols.add(g["ln1_b"][l].reshape(DT, P).T)
        idx[f"g2{l}"] = cols.add(g["ln2_g"][l].reshape(DT, P).T)
        idx[f"be2{l}"] = cols.add(g["ln2_b"][l].reshape(DT, P).T)
        idx[f"pb{l}"] = cols.add(np.tile(g["pbias"][l][None, :], (P, 1)))
    idx["gf"] = cols.add(g["lnf_g"].reshape(DT, P).T)
    idx["bef"] = cols.add(g["lnf_b"].reshape(DT, P).T)
    idx["bout"] = cols.add(np.full((P, 1), g["bout"][0], f32))
    idx["eps"] = cols.add(np.full((P, 1), EPS, f32))
    idx["zero"] = cols.add(np.zeros((P, 1), f32))
    bcols = cols.array()
    idx["_nbc"] = bcols.shape[1]

    import ml_dtypes

    bf = lambda a: np.ascontiguousarray(a).astype(ml_dtypes.bfloat16)
    tr = lambda w: w.transpose(0, 2, 1)
    shared = {
        "bcols": bcols,
        "WmT": np.ascontiguousarray(g["Wm"].T),
        "WaT": np.ascontiguousarray(g["Wa"].T),
        "Wq_g": bf(_group_kxm(tr(g["Wq"]), DT, DT)),
        "Wk_g": bf(_group_kxm(tr(g["Wk"]), DT, DT)),
        "WvT": bf(tr(g["Wv"])),  # rhs-moving, row slabs
        "Wo_g": bf(_group_kxm(tr(g["Wo"]), DT, DT)),
        "W1_g": bf(_group_kxm(tr(g["W1"]), DT, FT)),
        "W2_g": bf(_group_kxm(tr(g["W2"]), FT, DT)),
        "WoutT": np.ascontiguousarray(g["Wout"].T),
        "onesb": np.ones((P, P), f32),
    }

    per_core = []
    peT_full = np.ascontiguousarray(g["pe"][0].T)
    for c in range(NCORES):
        b, chunk = c // GPC, c % GPC
        r0 = chunk * T
        per_core.append(
            {
                "mktT": np.ascontiguousarray(g["market_data"][b, r0 : r0 + T, :].T),
                "astT": np.ascontiguousarray(g["astro_data"][b, r0 : r0 + T, :].T),
                "peT": np.ascontiguousarray(peT_full[:, r0 : r0 + T]),
            }
        )
    return shared, per_core, idx


# ---------------------------------------------------------------- device kernel
def _build(idx):
    from contextlib import ExitStack

    import concourse.mybir as mybir
    import concourse.tile as tile
    from concourse import bacc

    dt = mybir.dt
    F32, F32R, BF16 = dt.float32, dt.float32r, dt.bfloat16
    AF = mybir.ActivationFunctionType
    ALU = mybir.AluOpType

    nc = bacc.Bacc("TRN2", debug=False, num_devices=NCORES)

    NBC = idx["_nbc"]

    mktT = nc.declare_dram_parameter("mktT", [NM, T], F32R, isOutput=False)
    astT = nc.declare_dram_parameter("astT", [NA, T], F32R, isOutput=False)
    peT = nc.declare_dram_parameter("peT", [D, T], F32, isOutput=False)
    bcols_d = nc.declare_dram_parameter("bcols", [P, NBC], F32, isOutput=False)
    WmT = nc.declare_dram_parameter("WmT", [NM, D // 2], F32R, isOutput=False)
    WaT = nc.declare_dram_parameter("WaT", [NA, D // 2], F32R, isOutput=False)
    Wq_g = nc.declare_dram_parameter("Wq_g", [L * DT, P, DT * P], BF16, isOutput=False)
    Wk_g = nc.declare_dram_parameter("Wk_g", [L * DT, P, DT * P], BF16, isOutput=False)
    WvT = nc.declare_dram_parameter("WvT", [L, D, D], BF16, isOutput=False)
    Wo_g = nc.declare_dram_parameter("Wo_g", [L * DT, P, DT * P], BF16, isOutput=False)
    W1_g = nc.declare_dram_parameter("W1_g", [L * FT, P, DT * P], BF16, isOutput=False)
    W2_g = nc.declare_dram_parameter("W2_g", [L * DT, P, FT * P], BF16, isOutput=False)
    WoutT = nc.declare_dram_parameter("WoutT", [D, OUT], F32R, isOutput=False)
    ones_d = nc.declare_dram_parameter("onesb", [P, P], F32R, isOutput=False)
    y_out = nc.declare_dram_parameter("y", [1, T], F32, isOutput=True)

    k_in = [nc.dram_tensor(f"k_in{l}", [KELEM], BF16) for l in range(L)]
    k_ag = [nc.dram_tensor(f"k_ag{l}", [GPC, KELEM], BF16) for l in range(L)]
    v_in = [nc.dram_tensor(f"v_in{l}", [VELEM], BF16) for l in range(L)]
    v_ag = [nc.dram_tensor(f"v_ag{l}", [GPC, VELEM], BF16) for l in range(L)]

    with tile.TileContext(nc) as tc, ExitStack() as ctx:
        def pool(name, bufs, space="SBUF"):
            return ctx.enter_context(tc.tile_pool(name=name, bufs=bufs, space=space))

        singles = pool("singles", 1)
        xp = pool("xarr", 3)
        xbp = pool("xbf", 2)
        qp = pool("qarr", 1)
        cxp = pool("ctxarr", 1)
        hp = pool("harr", 1)
        wp = pool("wrow", 12)  # qkvo/W1 k-groups + WvT row slabs (bf16)
        w2p = pool("w2grp", 4)  # (128,4096) bf16
        kvp = pool("kvloc", 2)
        kp1 = pool("kloc1", 1)
        kvap = pool("kvall", 1)
        exp_p = pool("exparr", 4)
        sqp = pool("sqp", 2)
        bcp = pool("bcp", 1)
        lntp = pool("lntp", 4)
        sp = pool("small", 3)
        embp = pool("embp", 2)
        nrmp = pool("nrmp", 2)
        nrm1 = pool("nrm1", 1)

        ps = pool("ps", 5, space="PSUM")
        psx = pool("psx", 3, space="PSUM")

        bc = singles.tile([P, NBC], F32)
        nc.sync.dma_start(bc[:], bcols_d[:])
        onesb = singles.tile([P, P], F32R)
        nc.sync.dma_start(onesb[:], ones_d[:])
        pe_sb = singles.tile([P, DT * T], F32)
        nc.sync.dma_start(pe_sb[:].rearrange("p (a t) -> p a t", t=T), peT[:].rearrange("(a p) t -> p a t", p=P))
        wout_sb = singles.tile([P, DT], F32R)
        nc.sync.dma_start(wout_sb[:].rearrange("p (a o) -> p a o", o=OUT), WoutT[:].rearrange("(a p) o -> p a o", p=P))

        def col(name, j=0, rows=P):
            return bc[0:rows, idx[name] + j : idx[name] + j + 1]

        def mm(out, lhsT, rhs, start, stop):
            nc.tensor.matmul(out, lhsT, rhs, start=start, stop=stop)

        for _rep in range(REPS):
            # ======================================================== embed
            x = xp.tile([P, DT * T], F32R, tag="xarr")
            xb = xbp.tile([P, DT * T], BF16, tag="xbf", name="xb_emb")
            in_sb = embp.tile([NA, 2 * T], F32R, tag="embin")
            nc.sync.dma_start(in_sb[0:NM, 0:T], mktT[:])
            nc.sync.dma_start(in_sb[0:NA, T : 2 * T], astT[:])
            wemb = embp.tile([NA, D // 2], F32R, tag="wemb")
            nc.sync.dma_start(wemb[0:NM, :], WmT[:])
            wemb2 = embp.tile([NA, D // 2], F32R, tag="wemb")
            nc.sync.dma_start(wemb2[:], WaT[:])
            for m in range(DT):
                pm = ps.tile([P, T], F32, tag="ps")
                if m < 4:
                    w, nin, toff, mo = wemb, NM, 0, m
                else:
                    w, nin, toff, mo = wemb2, NA, T, m - 4
                mm(pm[:], w[0:nin, mo * P : (mo + 1) * P], in_sb[0:nin, toff : toff + T],
                   start=True, stop=True)
                nc.vector.scalar_tensor_tensor(
                    x[:, m * T : (m + 1) * T], pm[:], col("bemb", m),
                    pe_sb[:, m * T : (m + 1) * T], ALU.add, ALU.add,
                )
                nc.gpsimd.tensor_scalar_mul(
                    xb[:, m * T : (m + 1) * T], x[:, m * T : (m + 1) * T], 1.0
                )

            # ============================================================ helpers
            def load_group(pl, src_row, ntiles, tag):
                t = pl.tile([P, ntiles * P], BF16, tag=tag)
                nc.sync.dma_start(t[:], src_row)
                return t

            def proj_fm(wg_d, row0, src, bias_fn, dst_fn, nm=DT, nk=DT):
                """Feature-major projection: dst[m] = act(W @ src + bias)."""
                for m in range(nm):
                    grp = load_group(w2p if nk == FT else wp, wg_d[row0 + m, :, :], nk,
                                     "w2grp" if nk == FT else "wrow")
                    pr = ps.tile([P, T], F32, tag="ps")
                    for kk in range(nk):
                        mm(pr[:], grp[:, kk * P : (kk + 1) * P],
                           src(kk), start=(kk == 0), stop=(kk == nk - 1))
                    dst_fn(m, pr, bias_fn(m))

            def layernorm(src_tiles, gname, bname, dst, dst_bf=None):
                """dst = LN(src) with gamma/beta; Act engine: only Ln+Exp."""
                s_ps = psx.tile([1, T], F32, tag="aux")
                s2_ps = psx.tile([1, T], F32, tag="aux")
                for m in range(DT):
                    eng = nc.gpsimd if m % 2 else nc.vector
                    sq = sqp.tile([P, T], F32R, tag="sq")
                    eng.tensor_mul(sq[:], src_tiles[m], src_tiles[m])
                    mm(s2_ps[:], onesb[:, 0:1], sq[:], start=(m == 0), stop=(m == DT - 1))
                for m in range(DT):
                    mm(s_ps[:], onesb[:, 0:1], src_tiles[m], start=(m == 0), stop=(m == DT - 1))
                mu = sp.tile([1, T], F32R, tag="stat1")
                nc.vector.tensor_scalar_mul(mu[:], s_ps[:], 1.0 / D)
                ex2 = sp.tile([1, T], F32, tag="stat1")
                nc.vector.tensor_scalar_mul(ex2[:], s2_ps[:], 1.0 / D)
                mu2 = sp.tile([1, T], F32, tag="stat1")
                nc.vector.tensor_mul(mu2[:], mu[:], mu[:])
                var = sp.tile([1, T], F32, tag="stat1")
                nc.vector.scalar_tensor_tensor(var[:], mu2[:], -1.0, ex2[:],
                                               ALU.mult, ALU.add)
                # rs = rsqrt(var+eps) = exp(-0.5*ln(var+eps)): stays in the
                # exp/ln act table — no act-func-set reloads anywhere.
                lv = sp.tile([1, T], F32, tag="stat1")
                nc.scalar.activation(lv[:], var[:], AF.Ln,
                                     bias=col("eps", rows=1), scale=1.0)
                rs = sp.tile([1, T], F32R, tag="stat1")
                with nc.allow_low_precision(reason="fp32r feeds the broadcast matmul"):
                    nc.scalar.activation(rs[:], lv[:], AF.Exp,
                                         bias=col("zero", rows=1), scale=-0.5)
                mub_ps = psx.tile([P, T], F32, tag="aux")
                mm(mub_ps[:], onesb[0:1, :], mu[:], start=True, stop=True)
                rsb_ps = psx.tile([P, T], F32, tag="aux")
                mm(rsb_ps[:], onesb[0:1, :], rs[:], start=True, stop=True)
                rsb = bcp.tile([P, T], F32, tag="bcast")
                nc.scalar.copy(rsb[:], rsb_ps[:])
                for m in range(DT):
                    eng = nc.vector if m % 2 else nc.gpsimd
                    eng2 = nc.gpsimd if m % 2 else nc.vector
                    t1 = lntp.tile([P, T], F32, tag="lnt")
                    nc.vector.tensor_sub(t1[:], src_tiles[m], mub_ps[:])
                    t2 = lntp.tile([P, T], F32, tag="lnt")
                    eng.tensor_mul(t2[:], t1[:], rsb[:])
                    eng.tensor_scalar(
                        dst[:, m * T : (m + 1) * T], t2[:], col(gname, m), col(bname, m),
                        ALU.mult, ALU.add,
                    )
                    if dst_bf is not None:
                        eng2.tensor_scalar(
                            dst_bf[:, m * T : (m + 1) * T], t2[:],
                            col(gname, m), col(bname, m), ALU.mult, ALU.add,
                        )

            # ============================================================ layers
            for l in range(L):
                # ---- k projection (feature-major) -> k_in bf16; AG(K) ASAP
                karr = kp1.tile([P, DT * T], BF16, tag="kloc", name=f"karr{l}")

                def k_dst(m, pr, bias, karr=karr, l=l):
                    nc.vector.tensor_scalar_add(karr[:, m * T : (m + 1) * T], pr[:], bias)
                    nc.sync.dma_start(
                        k_in[l][:].rearrange("(r t) -> r t", t=T)[m * P : (m + 1) * P, :],
                        karr[:, m * T : (m + 1) * T],
                    )

                proj_fm(Wk_g, l * DT, lambda kk, xb=xb: xb[:, kk * T : (kk + 1) * T],
                        lambda m, l=l: col(f"bk{l}", m), k_dst)
                nc.gpsimd.collective_compute(
                    "AllGather", ALU.bypass, replica_groups=GROUPS,
                    ins=[k_in[l][:].opt()], outs=[k_ag[l][:].opt()],
                )

                # ---- v projection (token-major, per-head [v 64 | one] rows)
                vls = []
                for mt in range(TT):
                    vl = kvp.tile([P, VROW], BF16, tag="vloc")
                    # ones column at [h*65 + 64]
                    nc.vector.memset(
                        vl[:].rearrange("p (h c) -> p h c", c=PV)[:, :, HD:PV], 1.0,
                    )
                    vls.append(vl)
                for n in range(2):
                    pvs = [ps.tile([P, 512], F32, tag="ps", name=f"pv{l}_{n}_{mt}")
                           for mt in range(TT)]
                    for kk in range(DT):
                        slab = wp.tile([P, 512], BF16, tag="wrow")
                        nc.sync.dma_start(
                            slab[:], WvT[l, kk * P : (kk + 1) * P, n * 512 : (n + 1) * 512]
                        )
                        for mt in range(TT):
                            mm(pvs[mt][:], xb[:, kk * T + mt * P : kk * T + (mt + 1) * P],
                               slab[:], start=(kk == 0), stop=(kk == DT - 1))
                    for mt in range(TT):
                        # psum heads 8n..8n+7 -> per-head 65-col slots
                        vl8 = vls[mt][:].rearrange("p (h c) -> p h c", c=PV)[
                            :, 8 * n : 8 * (n + 1), 0:HD
                        ]
                        nc.vector.tensor_scalar_mul(
                            vl8, pvs[mt][:].rearrange("p (h d) -> p h d", d=HD), 1.0,
                        )
                for mt in range(TT):
                    nc.sync.dma_start(
                        v_in[l][:].rearrange("(r t) -> r t", t=VROW)[mt * P : (mt + 1) * P, :],
                        vls[mt][:],
                    )
                nc.gpsimd.collective_compute(
                    "AllGather", ALU.bypass, replica_groups=GROUPS,
                    ins=[v_in[l][:].opt()], outs=[v_ag[l][:].opt()],
                )

                # ---- q projection (feature-major, pre-scaled by 1/8); overlaps AGs
                q = qp.tile([P, DT * T], BF16, tag="qarr")

                def q_dst(m, pr, bias, q=q):
                    nc.vector.tensor_scalar(
                        q[:, m * T : (m + 1) * T], pr[:], 0.125, bias, ALU.mult, ALU.add
                    )

                proj_fm(Wq_g, l * DT, lambda kk, xb=xb: xb[:, kk * T : (kk + 1) * T],
                        lambda m, l=l: col(f"bq{l}", m), q_dst)

                # ---- bulk-load gathered K/V into SBUF
                k_all = kvap.tile([P, GPC * DT * T], BF16, tag="kall", name=f"kall{l}")
                v_all = kvap.tile([P, GPC * TT * VROW], BF16, tag="vall", name=f"vall{l}")
                for c in range(GPC):
                    nc.sync.dma_start(
                        k_all[:, c * DT * T : (c + 1) * DT * T].rearrange(
                            "p (a t) -> p a t", t=T),
                        k_ag[l][c, :].rearrange("(a p t) -> p a t", p=P, t=T),
                    )
                for c in range(GPC):
                    nc.gpsimd.dma_start(
                        v_all[:, c * TT * VROW : (c + 1) * TT * VROW].rearrange(
                            "p (a v) -> p a v", v=VROW),
                        v_ag[l][c, :].rearrange("(a p v) -> p a v", p=P, v=VROW),
                    )

                # ---- attention, one head pair per q partition tile.
                # Emit order S(0) S(1) C(0) S(2) C(1) ... so the tensor queue
                # never blocks on exp more than one pair ahead.
                ctxa = cxp.tile([P, DT * T], BF16, tag="ctxarr")
                eas = [None] * NPAIR

                def scores_phase(p):
                    ea = exp_p.tile([P, ST * T], BF16, tag="exparr")
                    eb = exp_p.tile([P, ST * T], BF16, tag="exparr")
                    eas[p] = (ea, eb)
                    for c in range(GPC):
                        for h01 in range(2):
                            o = h01 * HD
                            pscore = ps.tile([P, 2 * T], F32, tag="ps")
                            for sub in range(TT):
                                kcol = c * DT * T + p * T + sub * P
                                mm(pscore[:, sub * T : (sub + 1) * T],
                                   k_all[o : o + HD, kcol : kcol + P],
                                   q[o : o + HD, p * T : (p + 1) * T], start=True, stop=True)
                            dst = ea if h01 == 0 else eb
                            nc.scalar.activation(
                                dst[:, TT * c * T : TT * (c + 1) * T], pscore[:], AF.Exp,
                                bias=col(f"pb{l}", 2 * p + h01), scale=1.0,
                            )

                def ctx_phase(p):
                    ea, eb = eas[p]
                    # two psum banks (interleaved accumulation chains must not
                    # share a bank): h0 -> A, h1 -> B, dens at partition 64
                    pca = psx.tile([P, T], F32, tag="aux")
                    pcb = psx.tile([P, T], F32, tag="aux")
                    for tk in range(ST):
                        c, sub = tk // TT, tk % TT
                        vcol = c * TT * VROW + sub * VROW
                        mm(pca[0:PV, :],
                           v_all[:, vcol + 2 * p * PV : vcol + (2 * p + 1) * PV],
                           ea[:, tk * T : (tk + 1) * T],
                           start=(tk == 0), stop=(tk == ST - 1))
                        mm(pcb[0:PV, :],
                           v_all[:, vcol + (2 * p + 1) * PV : vcol + (2 * p + 2) * PV],
                           eb[:, tk * T : (tk + 1) * T],
                           start=(tk == 0), stop=(tk == ST - 1))
                    den = nrm1.tile([1, 2 * T], F32, tag="den")
                    nc.vector.tensor_scalar_mul(den[0:1, 0:T], pca[HD : HD + 1, :], 1.0)
                    nc.vector.tensor_scalar_mul(den[0:1, T : 2 * T], pcb[HD : HD + 1, :], 1.0)
                    rec = nrm1.tile([1, 2 * T], F32R, tag="rec")
                    with nc.allow_low_precision(reason="fp32r feeds the broadcast matmul"):
                        nc.vector.reciprocal(rec[:], den[:])
                    pbc = ps.tile([P, 2 * T], F32, tag="ps")
                    mm(pbc[:], onesb[0:1, :], rec[0:1, :], start=True, stop=True)
                    bcsb = nrmp.tile([P, 2 * T], F32, tag="bc128")
                    nc.vector.tensor_scalar_mul(bcsb[:], pbc[:], 1.0)
                    nc.vector.tensor_mul(ctxa[0:HD, p * T : (p + 1) * T],
                                         pca[0:HD, :], bcsb[0:HD, 0:T])
                    nc.vector.tensor_mul(ctxa[HD : 2 * HD, p * T : (p + 1) * T],
                                         pcb[0:HD, :], bcsb[0:HD, T : 2 * T])

                scores_phase(0)
                for p in range(1, NPAIR):
                    scores_phase(p)
                    ctx_phase(p - 1)
                ctx_phase(NPAIR - 1)

                # ---- out projection + residual + LN1
                x1p = xp.tile([P, DT * T], F32R, tag="xarr")

                def o_dst(m, pr, bias, x1p=x1p, x=x):
                    nc.vector.scalar_tensor_tensor(
                        x1p[:, m * T : (m + 1) * T], pr[:], bias,
                        x[:, m * T : (m + 1) * T], ALU.add, ALU.add,
                    )

                proj_fm(Wo_g, l * DT, lambda kk: ctxa[:, kk * T : (kk + 1) * T],
                        lambda m, l=l: col(f"bo{l}", m), o_dst)
                x1 = xp.tile([P, DT * T], F32R, tag="xarr")
                x1b = xbp.tile([P, DT * T], BF16, tag="xbf", name=f"x1b_{l}")
                layernorm([x1p[:, m * T : (m + 1) * T] for m in range(DT)],
                          f"g1{l}", f"be1{l}", x1, dst_bf=x1b)

                # ---- FFN
                harr = hp.tile([P, FT * T], BF16, tag="harr")

                def h_dst(mf, pr, bias, harr=harr):
                    nc.vector.tensor_scalar(
                        harr[:, mf * T : (mf + 1) * T], pr[:], bias, 0.0, ALU.add, ALU.max
                    )

                proj_fm(W1_g, l * FT, lambda kk, x1b=x1b: x1b[:, kk * T : (kk + 1) * T],
                        lambda mf, l=l: col(f"b1{l}", mf), h_dst, nm=FT)

                x2p = xp.tile([P, DT * T], F32R, tag="xarr")

                def y_dst(m, pr, bias, x2p=x2p, x1=x1):
                    nc.vector.scalar_tensor_tensor(
                        x2p[:, m * T : (m + 1) * T], pr[:], bias,
                        x1[:, m * T : (m + 1) * T], ALU.add, ALU.add,
                    )

                proj_fm(W2_g, l * DT, lambda kf: harr[:, kf * T : (kf + 1) * T],
                        lambda m, l=l: col(f"b2{l}", m), y_dst, nk=FT)
                x2 = xp.tile([P, DT * T], F32R, tag="xarr")
                if l < L - 1:
                    xb = xbp.tile([P, DT * T], BF16, tag="xbf", name=f"xb_{l + 1}")
                else:
                    xb = None
                layernorm([x2p[:, m * T : (m + 1) * T] for m in range(DT)],
                          f"g2{l}", f"be2{l}", x2, dst_bf=xb)
                x = x2

            # ============================================================ head
            xf = xp.tile([P, DT * T], F32R, tag="xarr")
            layernorm([x[:, m * T : (m + 1) * T] for m in range(DT)], "gf", "bef", xf)
            pyf = psx.tile([1, T], F32, tag="aux")
            for m in range(DT):
                mm(pyf[:], wout_sb[:, m : m + 1], xf[:, m * T : (m + 1) * T],
                   start=(m == 0), stop=(m == DT - 1))
            ysb = sp.tile([1, T], F32, tag="stat1")
            nc.vector.tensor_scalar_add(ysb[:], pyf[:], col("bout", 0, rows=1))
            nc.sync.dma_start(y_out[:], ysb[:])

    nc.compile()
    return nc


# ---------------------------------------------------------------- runner
_SHARED_NAMES = frozenset(
    ["bcols", "WmT", "WaT", "Wq_g", "Wk_g", "WvT", "Wo_g", "W1_g", "W2_g",
     "WoutT", "onesb"]
)


def _make_runner(nc):
    import jax
    import jax.numpy as jnp
    import concourse.mybir as mybir
    from concourse import bass2jax
    from jax.sharding import Mesh, PartitionSpec
    from jax.experimental.shard_map import shard_map

    bass2jax.install_neuronx_cc_hook()

    partition_name = nc.partition_id_tensor.name if nc.partition_id_tensor else None
    in_names, out_names, out_avals = [], [], []
    for alloc in nc.m.functions[0].allocations:
        if not isinstance(alloc, mybir.MemoryLocationSet):
            continue
        name = alloc.memorylocations[0].name
        if alloc.kind == "ExternalInput":
            if name != partition_name:
                in_names.append(name)
        elif alloc.kind == "ExternalOutput":
            out_names.append(name)
            out_avals.append(
                jax.core.ShapedArray(tuple(alloc.tensor_shape), mybir.dt.np(alloc.dtype))
            )
    n_params = len(in_names)
    all_in = in_names + out_names + ([partition_name] if partition_name else [])

    def _body(*args):
        operands = list(args)
        if partition_name is not None:
            operands.append(bass2jax.partition_id_tensor())
        return tuple(
            bass2jax._bass_exec_p.bind(
                *operands,
                out_avals=tuple(out_avals),
                in_names=tuple(all_in),
                out_names=tuple(out_names),
                lowering_input_output_aliases=(),
                sim_require_finite=True,
                sim_require_nnan=True,
                nc=nc,
            )
        )

    from jax.sharding import NamedSharding

    devices = jax.devices()[:NCORES]
    mesh = Mesh(np.asarray(devices), ("core",))
    repl_sharding = NamedSharding(mesh, PartitionSpec(None))
    core_sharding = NamedSharding(mesh, PartitionSpec("core"))
    in_specs = tuple(
        PartitionSpec(None) if name in _SHARED_NAMES else PartitionSpec("core")
        for name in in_names
    ) + (PartitionSpec("core"),) * len(out_avals)
    out_specs = (PartitionSpec("core"),) * len(out_avals)
    sharded = jax.jit(
        shard_map(_body, mesh=mesh, in_specs=in_specs, out_specs=out_specs,
                  check_rep=False),
        keep_unused=True,
    )

    class Runner:
        def upload(self, shared, per_core):
            ins = []
            for name in in_names:
                if name in _SHARED_NAMES:
                    ins.append((np.asarray(shared[name]), repl_sharding))
                else:
                    ins.append(
                        (
                            np.concatenate(
                                [np.asarray(per_core[c][name])
                                 for c in range(NCORES)],
                                axis=0,
                            ),
                            core_sharding,
                        )
                    )
            # zero output buffers: uploaded once, reused every dispatch
            # (no donation, so they are never consumed)
            for av in out_avals:
                z = np.zeros((NCORES * av.shape[0], *av.shape[1:]), av.dtype)
                ins.append((z, core_sharding))
            self.in_dev = [jax.device_put(a, s) for a, s in ins]
            jax.block_until_ready(self.in_dev)

        def dispatch(self):
            return sharded(*self.in_dev)

        def collect(self, out_arrs):
            return [
                {
                    name: np.asarray(out_arrs[i]).reshape(NCORES, *out_avals[i].shape)[c]
                    for i, name in enumerate(out_names)
                }
                for c in range(NCORES)
            ]

        def run(self):
            import jax as _jax

            out_arrs = self.dispatch()
            _jax.block_until_ready(out_arrs)
            return self.collect(out_arrs)

    return Runner()


def get_runner(inputs):
    """Build (once) and return the runner with inputs uploaded."""
    global _RUNNER
    shared, per_core, idx = _prep_host(inputs)
    if _RUNNER is None:
        nc = _build(idx)
        _RUNNER = _make_runner(nc)
    _RUNNER.upload(shared, per_core)
    return _RUNNER


def kernel(**inputs) -> np.ndarray:
    runner = get_runner(inputs)
    res = runner.run()
    out = np.zeros((B, S, OUT), np.float32)
    for c in range(NCORES):
        b, chunk = c // GPC, c % GPC
        out[b, chunk * T : (chunk + 1) * T, 0] = res[c]["y"][0]
    return out


# revision 41
# speedup vs baseline: 2.6841x; 1.0006x over previous
"""AstroEconomicTransformer on 8 Trainium2 NeuronCores.

Sharding: 8-way sequence-parallel over the B*S = 2048 tokens (256 tokens
per core; cores 0-3 hold batch 0, cores 4-7 batch 1). Activations live
feature-major on chip (x^T: features on partitions, tokens on the free
dim), so every linear layer is a W^T-stationary matmul with the token
dim streaming. Attention needs the full sequence of K/V per batch
element: each layer runs TWO bf16 AllGathers (K right after the K
projection, V after the V projection) within each group of 4 cores, so
the V projection, Q projection and the first score matmuls overlap the
collectives; everything else (LayerNorm, FFN, projections, softmax) is
token-local.

Attention per head-pair: scores^T = k^T.T @ q^T (keys on partitions,
queries free), exp on the scalar engine with the 1/8 scale folded into
the q eviction and pbias as the activation bias (no max subtraction:
scores are O(1) by construction). Token-major V is laid out per head as
[v 64 | one] so the 65-column ctx matmul emits the softmax denominator
at psum partition 64 of the same accumulation — no separate denominator
matmuls. Both heads of a pair accumulate into one psum bank (h0 in
columns 0:256, h1 in 256:512), so one (1,512) reciprocal covers both
denominators; normalization broadcasts the reciprocals with a K=1
matmul. LayerNorm's rsqrt runs as exp(-0.5*ln(var+eps)) so the Act
engine never leaves the exp/ln table. The V bias is folded on host into
the attention output bias (bo_eff = bo + Wo @ bv, exact because softmax
rows sum to one).
"""

import numpy as np

B, S = 2, 1024
D, H, L, DFF = 1024, 16, 6, 4096
NM, NA, OUT = 10, 20, 1
HD = D // H
EPS = 1e-5

NCORES = 8
GPC = 4  # cores per batch group
T = (B * S) // NCORES  # 256 tokens per core
GROUPS = [[0, 1, 2, 3], [4, 5, 6, 7]]
P = 128
DT = D // P  # 8 feature tiles
FT = DFF // P  # 32 dff tiles
TT = T // P  # 2 token tiles per core
ST = S // P  # 8 key tiles per sequence
NPAIR = H // 2

PV = HD + 1  # 65: per-head v row [v 64 | one]
VROW = H * PV  # 1040
KELEM = D * T  # 262144
VELEM = T * VROW  # 266240

_RUNNER = None
REPS = 1


class _Cols:
    """Allocates columns in the (128, n) bias/constant matrix."""

    def __init__(self):
        self.cols = []

    def add(self, mat):  # mat: (128, n) -> first col index
        i = len(self.cols)
        self.cols.extend(np.asarray(mat, np.float32).T)
        return i

    def array(self):
        return np.stack(self.cols, axis=1).astype(np.float32)


def _group_kxm(wT, km, mm_):
    """(N, K, M) pre-transposed weight -> (N*mm_, P, km*P) SBUF tile images."""
    n = wT.shape[0]
    g = wT.reshape(n, km, P, mm_, P).transpose(0, 3, 2, 1, 4)
    return np.ascontiguousarray(g.reshape(n * mm_, P, km * P))


def _make_selb():
    s = np.zeros((P, 4 * P), np.float32)
    for j in range(4):
        s[32 * j, j * P : (j + 1) * P] = 1.0
    return s


def _prep_host(inputs):
    f32 = np.float32
    g = {k: np.asarray(v, f32) for k, v in inputs.items()}

    cols = _Cols()
    idx = {}
    bemb = np.concatenate([g["bm"], g["ba"]])
    idx["bemb"] = cols.add(bemb.reshape(DT, P).T)
    for l in range(L):
        idx[f"bq{l}"] = cols.add(g["bq"][l].reshape(DT, P).T)
        idx[f"bk{l}"] = cols.add(g["bk"][l].reshape(DT, P).T)
        bo_eff = g["bo"][l] + g["Wo"][l] @ g["bv"][l]
        idx[f"bo{l}"] = cols.add(bo_eff.reshape(DT, P).T)
        idx[f"b1{l}"] = cols.add(g["b1"][l].reshape(FT, P).T)
        idx[f"b2{l}"] = cols.add(g["b2"][l].reshape(DT, P).T)
        idx[f"g1{l}"] = cols.add(g["ln1_g"][l].reshape(DT, P).T)
        idx[f"be1{l}"] = cols.add(g["ln1_b"][l].reshape(DT, P).T)
        idx[f"g2{l}"] = cols.add(g["ln2_g"][l].reshape(DT, P).T)
        idx[f"be2{l}"] = cols.add(g["ln2_b"][l].reshape(DT, P).T)
        idx[f"pb{l}"] = cols.add(np.tile(g["pbias"][l][None, :], (P, 1)))
    idx["gf"] = cols.add(g["lnf_g"].reshape(DT, P).T)
    idx["bef"] = cols.add(g["lnf_b"].reshape(DT, P).T)
    idx["bout"] = cols.add(np.full((P, 1), g["bout"][0], f32))
    idx["eps"] = cols.add(np.full((P, 1), EPS, f32))
    idx["zero"] = cols.add(np.zeros((P, 1), f32))
    bcols = cols.array()
    idx["_nbc"] = bcols.shape[1]

    import ml_dtypes

    bf = lambda a: np.ascontiguousarray(a).astype(ml_dtypes.bfloat16)
    tr = lambda w: w.transpose(0, 2, 1)
    shared = {
        "bcols": bcols,
        "WmT": np.ascontiguousarray(g["Wm"].T),
        "WaT": np.ascontiguousarray(g["Wa"].T),
        "Wq_g": bf(_group_kxm(tr(g["Wq"]), DT, DT)),
        "Wk_g": bf(_group_kxm(tr(g["Wk"]), DT, DT)),
        "WvT": bf(tr(g["Wv"])),  # rhs-moving, row slabs
        "Wo_g": bf(_group_kxm(tr(g["Wo"]), DT, DT)),
        "W1_g": bf(_group_kxm(tr(g["W1"]), DT, FT)),
        "W2_g": bf(_group_kxm(tr(g["W2"]), FT, DT)),
        "WoutT": np.ascontiguousarray(g["Wout"].T),
        "onesb": np.ones((P, P), f32),
        # selb[:, j*128:(j+1)*128].T @ rec broadcasts rec row 32*j to all
        # 128 partitions (dens live at 32-aligned partitions)
        "selb": _make_selb(),
    }

    per_core = []
    peT_full = np.ascontiguousarray(g["pe"][0].T)
    for c in range(NCORES):
        b, chunk = c // GPC, c % GPC
        r0 = chunk * T
        per_core.append(
            {
                "mktT": np.ascontiguousarray(g["market_data"][b, r0 : r0 + T, :].T),
                "astT": np.ascontiguousarray(g["astro_data"][b, r0 : r0 + T, :].T),
                "peT": np.ascontiguousarray(peT_full[:, r0 : r0 + T]),
            }
        )
    return shared, per_core, idx


# ---------------------------------------------------------------- device kernel
def _build(idx):
    from contextlib import ExitStack

    import concourse.mybir as mybir
    import concourse.tile as tile
    from concourse import bacc

    dt = mybir.dt
    F32, F32R, BF16 = dt.float32, dt.float32r, dt.bfloat16
    FP8 = dt.float8e4
    AF = mybir.ActivationFunctionType
    ALU = mybir.AluOpType

    nc = bacc.Bacc("TRN2", debug=False, num_devices=NCORES)

    NBC = idx["_nbc"]

    mktT = nc.declare_dram_parameter("mktT", [NM, T], F32R, isOutput=False)
    astT = nc.declare_dram_parameter("astT", [NA, T], F32R, isOutput=False)
    peT = nc.declare_dram_parameter("peT", [D, T], F32, isOutput=False)
    bcols_d = nc.declare_dram_parameter("bcols", [P, NBC], F32, isOutput=False)
    WmT = nc.declare_dram_parameter("WmT", [NM, D // 2], F32R, isOutput=False)
    WaT = nc.declare_dram_parameter("WaT", [NA, D // 2], F32R, isOutput=False)
    Wq_g = nc.declare_dram_parameter("Wq_g", [L * DT, P, DT * P], BF16, isOutput=False)
    Wk_g = nc.declare_dram_parameter("Wk_g", [L * DT, P, DT * P], BF16, isOutput=False)
    WvT = nc.declare_dram_parameter("WvT", [L, D, D], BF16, isOutput=False)
    Wo_g = nc.declare_dram_parameter("Wo_g", [L * DT, P, DT * P], BF16, isOutput=False)
    W1_g = nc.declare_dram_parameter("W1_g", [L * FT, P, DT * P], BF16, isOutput=False)
    W2_g = nc.declare_dram_parameter("W2_g", [L * DT, P, FT * P], BF16, isOutput=False)
    WoutT = nc.declare_dram_parameter("WoutT", [D, OUT], F32R, isOutput=False)
    ones_d = nc.declare_dram_parameter("onesb", [P, P], F32R, isOutput=False)
    sel_d = nc.declare_dram_parameter("selb", [P, 4 * P], F32R, isOutput=False)
    y_out = nc.declare_dram_parameter("y", [1, T], F32, isOutput=True)

    KH = KELEM // 2
    k_inA = [nc.dram_tensor(f"k_inA{l}", [KH], FP8) for l in range(L)]
    k_agA = [nc.dram_tensor(f"k_agA{l}", [GPC, KH], FP8) for l in range(L)]
    k_inB = [nc.dram_tensor(f"k_inB{l}", [KH], FP8) for l in range(L)]
    k_agB = [nc.dram_tensor(f"k_agB{l}", [GPC, KH], FP8) for l in range(L)]
    v_in = [nc.dram_tensor(f"v_in{l}", [VELEM], FP8) for l in range(L)]
    v_ag = [nc.dram_tensor(f"v_ag{l}", [GPC, VELEM], FP8) for l in range(L)]
    warm_in = nc.dram_tensor("warm_in", [256], BF16)
    warm_ag = nc.dram_tensor("warm_ag", [GPC, 256], BF16)

    with tile.TileContext(nc) as tc, ExitStack() as ctx:
        def pool(name, bufs, space="SBUF"):
            return ctx.enter_context(tc.tile_pool(name=name, bufs=bufs, space=space))

        singles = pool("singles", 1)
        xp = pool("xarr", 3)
        xbp = pool("xbf", 2)
        qp = pool("qarr", 1)
        cxp = pool("ctxarr", 1)
        hp = pool("harr", 1)
        wp = pool("wrow", 12)  # qkvo/W1 k-groups + WvT row slabs (bf16)
        w2p = pool("w2grp", 4)  # (128,4096) bf16
        kvp = pool("kvloc", 2)
        kp1 = pool("kloc1", 1)
        kvap = pool("kvall", 1)
        exp_p = pool("exparr", 6)
        sqp = pool("sqp", 2)
        bcp = pool("bcp", 1)
        lntp = pool("lntp", 4)
        sp = pool("small", 3)
        embp = pool("embp", 2)
        nrmp = pool("nrmp", 2)
        nrm1 = pool("nrm1", 1)

        ps = pool("ps", 5, space="PSUM")
        psx = pool("psx", 3, space="PSUM")

        # warm up the collective path while the embed inputs stream in
        nc.gpsimd.collective_compute(
            "AllGather", ALU.bypass, replica_groups=GROUPS,
            ins=[warm_in[:].opt()], outs=[warm_ag[:].opt()],
        )
        bc = singles.tile([P, NBC], F32)
        nc.sync.dma_start(bc[:], bcols_d[:])
        onesb = singles.tile([P, P], F32R)
        nc.sync.dma_start(onesb[:], ones_d[:])
        selb = singles.tile([P, 4 * P], F32R)
        nc.sync.dma_start(selb[:], sel_d[:])
        pe_sb = singles.tile([P, DT * T], F32)
        wout_sb = singles.tile([P, DT], F32R)

        def col(name, j=0, rows=P):
            return bc[0:rows, idx[name] + j : idx[name] + j + 1]

        def mm(out, lhsT, rhs, start, stop):
            nc.tensor.matmul(out, lhsT, rhs, start=start, stop=stop)

        for _rep in range(REPS):
            # ======================================================== embed
            x = xp.tile([P, DT * T], F32R, tag="xarr")
            xb = xbp.tile([P, DT * T], BF16, tag="xbf", name="xb_emb")
            in_sb = embp.tile([NA, 2 * T], F32R, tag="embin")
            nc.sync.dma_start(in_sb[0:NM, 0:T], mktT[:])
            nc.sync.dma_start(in_sb[0:NA, T : 2 * T], astT[:])
            wemb = embp.tile([NA, D // 2], F32R, tag="wemb")
            nc.sync.dma_start(wemb[0:NM, :], WmT[:])
            wemb2 = embp.tile([NA, D // 2], F32R, tag="wemb")
            nc.sync.dma_start(wemb2[:], WaT[:])
            if _rep == 0:
                nc.sync.dma_start(pe_sb[:].rearrange("p (a t) -> p a t", t=T),
                                  peT[:].rearrange("(a p) t -> p a t", p=P))
                nc.sync.dma_start(wout_sb[:].rearrange("p (a o) -> p a o", o=OUT),
                                  WoutT[:].rearrange("(a p) o -> p a o", p=P))
            for m in range(DT):
                pm = ps.tile([P, T], F32, tag="ps")
                if m < 4:
                    w, nin, toff, mo = wemb, NM, 0, m
                else:
                    w, nin, toff, mo = wemb2, NA, T, m - 4
                mm(pm[:], w[0:nin, mo * P : (mo + 1) * P], in_sb[0:nin, toff : toff + T],
                   start=True, stop=True)
                nc.vector.scalar_tensor_tensor(
                    x[:, m * T : (m + 1) * T], pm[:], col("bemb", m),
                    pe_sb[:, m * T : (m + 1) * T], ALU.add, ALU.add,
                )
                nc.gpsimd.tensor_scalar_mul(
                    xb[:, m * T : (m + 1) * T], x[:, m * T : (m + 1) * T], 1.0
                )

            # ============================================================ helpers
            def load_group(pl, src_row, ntiles, tag):
                t = pl.tile([P, ntiles * P], BF16, tag=tag)
                nc.sync.dma_start(t[:], src_row)
                return t

            def proj_fm(wg_d, row0, src, bias_fn, dst_fn, nm=DT, nk=DT, after_m=None):
                """Feature-major projection: dst[m] = act(W @ src + bias)."""
                for m in range(nm):
                    grp = load_group(w2p if nk == FT else wp, wg_d[row0 + m, :, :], nk,
                                     "w2grp" if nk == FT else "wrow")
                    pr = ps.tile([P, T], F32, tag="ps")
                    for kk in range(nk):
                        mm(pr[:], grp[:, kk * P : (kk + 1) * P],
                           src(kk), start=(kk == 0), stop=(kk == nk - 1))
                    dst_fn(m, pr, bias_fn(m))
                    if after_m and m in after_m:
                        after_m[m]()

            def layernorm(src_tiles, gname, bname, dst, dst_bf=None):
                """dst = LN(src) with gamma/beta; Act engine: only Ln+Exp."""
                s_ps = psx.tile([1, T], F32, tag="aux")
                s2_ps = psx.tile([1, T], F32, tag="aux")
                for m in range(DT):
                    eng = nc.gpsimd if m % 2 else nc.vector
                    sq = sqp.tile([P, T], F32R, tag="sq")
                    eng.tensor_mul(sq[:], src_tiles[m], src_tiles[m])
                    mm(s2_ps[:], onesb[:, 0:1], sq[:], start=(m == 0), stop=(m == DT - 1))
                for m in range(DT):
                    mm(s_ps[:], onesb[:, 0:1], src_tiles[m], start=(m == 0), stop=(m == DT - 1))
                mu = sp.tile([1, T], F32R, tag="stat1")
                nc.vector.tensor_scalar_mul(mu[:], s_ps[:], 1.0 / D)
                ex2 = sp.tile([1, T], F32, tag="stat1")
                nc.vector.tensor_scalar_mul(ex2[:], s2_ps[:], 1.0 / D)
                mu2 = sp.tile([1, T], F32, tag="stat1")
                nc.vector.tensor_mul(mu2[:], mu[:], mu[:])
                var = sp.tile([1, T], F32, tag="stat1")
                nc.vector.scalar_tensor_tensor(var[:], mu2[:], -1.0, ex2[:],
                                               ALU.mult, ALU.add)
                # rs = rsqrt(var+eps) = exp(-0.5*ln(var+eps)): stays in the
                # exp/ln act table — no act-func-set reloads anywhere.
                lv = sp.tile([1, T], F32, tag="stat1")
                nc.scalar.activation(lv[:], var[:], AF.Ln,
                                     bias=col("eps", rows=1), scale=1.0)
                rs = sp.tile([1, T], F32R, tag="stat1")
                with nc.allow_low_precision(reason="fp32r feeds the broadcast matmul"):
                    nc.scalar.activation(rs[:], lv[:], AF.Exp,
                                         bias=col("zero", rows=1), scale=-0.5)
                mub_ps = psx.tile([P, T], F32, tag="aux")
                mm(mub_ps[:], onesb[0:1, :], mu[:], start=True, stop=True)
                rsb_ps = psx.tile([P, T], F32, tag="aux")
                mm(rsb_ps[:], onesb[0:1, :], rs[:], start=True, stop=True)
                rsb = bcp.tile([P, T], F32, tag="bcast")
                nc.scalar.copy(rsb[:], rsb_ps[:])
                for m in range(DT):
                    eng = nc.vector if m % 2 else nc.gpsimd
                    eng2 = nc.gpsimd if m % 2 else nc.vector
                    t1 = lntp.tile([P, T], F32, tag="lnt")
                    nc.vector.tensor_sub(t1[:], src_tiles[m], mub_ps[:])
                    t2 = lntp.tile([P, T], F32, tag="lnt")
                    eng.tensor_mul(t2[:], t1[:], rsb[:])
                    eng.tensor_scalar(
                        dst[:, m * T : (m + 1) * T], t2[:], col(gname, m), col(bname, m),
                        ALU.mult, ALU.add,
                    )
                    if dst_bf is not None:
                        eng2.tensor_scalar(
                            dst_bf[:, m * T : (m + 1) * T], t2[:],
                            col(gname, m), col(bname, m), ALU.mult, ALU.add,
                        )

            # ============================================================ layers
            for l in range(L):
                # ---- k projection (feature-major) -> fp8; AG(K) in 2 halves
                karr = kp1.tile([P, DT * T], FP8, tag="kloc", name=f"karr{l}")

                def k_dst(m, pr, bias, karr=karr, l=l):
                    nc.vector.tensor_scalar_add(karr[:, m * T : (m + 1) * T], pr[:], bias)
                    dst = k_inA[l] if m < 4 else k_inB[l]
                    nc.sync.dma_start(
                        dst[:].rearrange("(r t) -> r t", t=T)[(m % 4) * P : (m % 4 + 1) * P, :],
                        karr[:, m * T : (m + 1) * T],
                    )

                def ag_ka(l=l):
                    nc.gpsimd.collective_compute(
                        "AllGather", ALU.bypass, replica_groups=GROUPS,
                        ins=[k_inA[l][:].opt()], outs=[k_agA[l][:].opt()],
                    )

                proj_fm(Wk_g, l * DT, lambda kk, xb=xb: xb[:, kk * T : (kk + 1) * T],
                        lambda m, l=l: col(f"bk{l}", m), k_dst, after_m={3: ag_ka})
                nc.gpsimd.collective_compute(
                    "AllGather", ALU.bypass, replica_groups=GROUPS,
                    ins=[k_inB[l][:].opt()], outs=[k_agB[l][:].opt()],
                )

                # ---- v projection (token-major, per-head [v 64 | one] rows)
                vls = []
                for mt in range(TT):
                    vl = kvp.tile([P, VROW], FP8, tag="vloc")
                    # ones column at [h*65 + 64]
                    nc.vector.memset(
                        vl[:].rearrange("p (h c) -> p h c", c=PV)[:, :, HD:PV], 1.0,
                    )
                    vls.append(vl)
                for n in range(2):
                    pvs = [ps.tile([P, 512], F32, tag="ps", name=f"pv{l}_{n}_{mt}")
                           for mt in range(TT)]
                    for kk in range(DT):
                        slab = wp.tile([P, 512], BF16, tag="wrow")
                        nc.sync.dma_start(
                            slab[:], WvT[l, kk * P : (kk + 1) * P, n * 512 : (n + 1) * 512]
                        )
                        for mt in range(TT):
                            mm(pvs[mt][:], xb[:, kk * T + mt * P : kk * T + (mt + 1) * P],
                               slab[:], start=(kk == 0), stop=(kk == DT - 1))
                    for mt in range(TT):
                        # psum heads 8n..8n+7 -> per-head 65-col slots
                        vl8 = vls[mt][:].rearrange("p (h c) -> p h c", c=PV)[
                            :, 8 * n : 8 * (n + 1), 0:HD
                        ]
                        nc.scalar.copy(
                            vl8, pvs[mt][:].rearrange("p (h d) -> p h d", d=HD),
                        )
                for mt in range(TT):
                    nc.sync.dma_start(
                        v_in[l][:].rearrange("(r t) -> r t", t=VROW)[mt * P : (mt + 1) * P, :],
                        vls[mt][:],
                    )
                nc.gpsimd.collective_compute(
                    "AllGather", ALU.bypass, replica_groups=GROUPS,
                    ins=[v_in[l][:].opt()], outs=[v_ag[l][:].opt()],
                )

                # ---- q projection (feature-major; the 1/8 scale is folded into
                # the exp activation's scale); overlaps AGs
                q = qp.tile([P, DT * T], FP8, tag="qarr")

                def q_dst(m, pr, bias, q=q):
                    nc.vector.tensor_scalar_add(q[:, m * T : (m + 1) * T], pr[:], bias)

                proj_fm(Wq_g, l * DT, lambda kk, xb=xb: xb[:, kk * T : (kk + 1) * T],
                        lambda m, l=l: col(f"bq{l}", m), q_dst)

                # ---- bulk-load gathered K/V into SBUF (A halves first: pairs
                # 0-3 score against tiles 0-3 only)
                k_all = kvap.tile([P, GPC * DT * T], FP8, tag="kall", name=f"kall{l}")
                v_all = kvap.tile([P, GPC * TT * VROW], FP8, tag="vall", name=f"vall{l}")
                HT = 4 * T
                for c in range(GPC):
                    nc.sync.dma_start(
                        k_all[:, c * DT * T : c * DT * T + HT].rearrange(
                            "p (a t) -> p a t", t=T),
                        k_agA[l][c, :].rearrange("(a p t) -> p a t", p=P, t=T),
                    )
                for c in range(GPC):
                    nc.sync.dma_start(
                        k_all[:, c * DT * T + HT : (c + 1) * DT * T].rearrange(
                            "p (a t) -> p a t", t=T),
                        k_agB[l][c, :].rearrange("(a p t) -> p a t", p=P, t=T),
                    )
                for c in range(GPC):
                    nc.gpsimd.dma_start(
                        v_all[:, c * TT * VROW : (c + 1) * TT * VROW].rearrange(
                            "p (a v) -> p a v", v=VROW),
                        v_ag[l][c, :].rearrange("(a p v) -> p a v", p=P, v=VROW),
                    )

                # ---- attention, one head pair per q partition tile.
                # Emit order S(0) S(1) C(0) S(2) C(1) ... so the tensor queue
                # never blocks on exp more than one pair ahead.
                ctxa = cxp.tile([P, DT * T], BF16, tag="ctxarr")
                ctxu = cxp.tile([P, DT * T], F32, tag="ctxu")
                stashes = []
                for g in range(2):
                    st = nrm1.tile([P, 2 * T], F32, tag=f"den{g}", name=f"stash{l}_{g}")
                    # unwritten lanes must stay finite through the reciprocal
                    nc.vector.memset(st[:], 1.0)
                    stashes.append(st)
                eas = [None] * NPAIR

                def scores_phase(p):
                    ea = exp_p.tile([P, ST * T], FP8, tag="exparr")
                    eb = exp_p.tile([P, ST * T], FP8, tag="exparr")
                    eas[p] = (ea, eb)
                    for c in range(GPC):
                        for h01 in range(2):
                            o = h01 * HD
                            pscore = ps.tile([P, 2 * T], F32, tag="ps")
                            for sub in range(TT):
                                kcol = c * DT * T + p * T + sub * P
                                mm(pscore[:, sub * T : (sub + 1) * T],
                                   k_all[o : o + HD, kcol : kcol + P],
                                   q[o : o + HD, p * T : (p + 1) * T], start=True, stop=True)
                            dst = ea if h01 == 0 else eb
                            nc.scalar.activation(
                                dst[:, TT * c * T : TT * (c + 1) * T], pscore[:], AF.Exp,
                                bias=col(f"pb{l}", 2 * p + h01), scale=0.125,
                            )

                def ctx_phase(p):
                    ea, eb = eas[p]
                    # two psum banks (interleaved accumulation chains must not
                    # share a bank): h0 -> A, h1 -> B, dens at partition 64
                    pca = psx.tile([P, T], F32, tag="aux")
                    pcb = psx.tile([P, T], F32, tag="aux")
                    for tk in range(ST):
                        c, sub = tk // TT, tk % TT
                        vcol = c * TT * VROW + sub * VROW
                        mm(pca[0:PV, :],
                           v_all[:, vcol + 2 * p * PV : vcol + (2 * p + 1) * PV],
                           ea[:, tk * T : (tk + 1) * T],
                           start=(tk == 0), stop=(tk == ST - 1))
                        mm(pcb[0:PV, :],
                           v_all[:, vcol + (2 * p + 1) * PV : vcol + (2 * p + 2) * PV],
                           eb[:, tk * T : (tk + 1) * T],
                           start=(tk == 0), stop=(tk == ST - 1))
                    st, j = stashes[p // 4], 32 * (p % 4)
                    nc.vector.tensor_scalar_mul(st[j : j + 1, 0:T],
                                                pca[HD : HD + 1, :], 1.0)
                    nc.vector.tensor_scalar_mul(st[j : j + 1, T : 2 * T],
                                                pcb[HD : HD + 1, :], 1.0)
                    nc.vector.tensor_scalar_mul(ctxu[0:HD, p * T : (p + 1) * T],
                                                pca[0:HD, :], 1.0)
                    nc.vector.tensor_scalar_mul(ctxu[HD : 2 * HD, p * T : (p + 1) * T],
                                                pcb[0:HD, :], 1.0)

                def norm_phase(grp):
                    # one reciprocal covers 4 pairs' denominators (rows 32j)
                    rec4 = nrm1.tile([P, 2 * T], F32R, tag=f"rec{grp}")
                    with nc.allow_low_precision(reason="fp32r feeds the broadcast matmul"):
                        nc.vector.reciprocal(rec4[:], stashes[grp][:])
                    for j in range(4):
                        p = 4 * grp + j
                        pbc = ps.tile([P, 2 * T], F32, tag="ps")
                        mm(pbc[:], selb[:, j * P : (j + 1) * P], rec4[:],
                           start=True, stop=True)
                        bcsb = nrmp.tile([P, 2 * T], F32, tag="bc128")
                        nc.vector.tensor_scalar_mul(bcsb[:], pbc[:], 1.0)
                        nc.gpsimd.tensor_mul(ctxa[0:HD, p * T : (p + 1) * T],
                                             ctxu[0:HD, p * T : (p + 1) * T],
                                             bcsb[0:HD, 0:T])
                        nc.gpsimd.tensor_mul(ctxa[HD : 2 * HD, p * T : (p + 1) * T],
                                             ctxu[HD : 2 * HD, p * T : (p + 1) * T],
                                             bcsb[HD : 2 * HD, T : 2 * T])

                scores_phase(0)
                for p in range(1, NPAIR):
                    scores_phase(p)
                    ctx_phase(p - 1)
                    if p == 4:
                        norm_phase(0)
                ctx_phase(NPAIR - 1)
                norm_phase(1)

                # ---- out projection + residual + LN1
                x1p = xp.tile([P, DT * T], F32R, tag="xarr")

                def o_dst(m, pr, bias, x1p=x1p, x=x):
                    nc.vector.scalar_tensor_tensor(
                        x1p[:, m * T : (m + 1) * T], pr[:], bias,
                        x[:, m * T : (m + 1) * T], ALU.add, ALU.add,
                    )

                proj_fm(Wo_g, l * DT, lambda kk: ctxa[:, kk * T : (kk + 1) * T],
                        lambda m, l=l: col(f"bo{l}", m), o_dst)
                x1 = xp.tile([P, DT * T], F32R, tag="xarr")
                x1b = xbp.tile([P, DT * T], BF16, tag="xbf", name=f"x1b_{l}")
                layernorm([x1p[:, m * T : (m + 1) * T] for m in range(DT)],
                          f"g1{l}", f"be1{l}", x1, dst_bf=x1b)

                # ---- FFN
                harr = hp.tile([P, FT * T], BF16, tag="harr")

                def h_dst(mf, pr, bias, harr=harr):
                    # relu(pr + b1) on the Act engine (Relu is in every table
                    # set); frees the vector engine during the FFN phase
                    nc.scalar.activation(
                        harr[:, mf * T : (mf + 1) * T], pr[:], AF.Relu, bias=bias,
                        scale=1.0,
                    )

                proj_fm(W1_g, l * FT, lambda kk, x1b=x1b: x1b[:, kk * T : (kk + 1) * T],
                        lambda mf, l=l: col(f"b1{l}", mf), h_dst, nm=FT)

                x2p = xp.tile([P, DT * T], F32R, tag="xarr")

                def y_dst(m, pr, bias, x2p=x2p, x1=x1):
                    nc.vector.scalar_tensor_tensor(
                        x2p[:, m * T : (m + 1) * T], pr[:], bias,
                        x1[:, m * T : (m + 1) * T], ALU.add, ALU.add,
                    )

                proj_fm(W2_g, l * DT, lambda kf: harr[:, kf * T : (kf + 1) * T],
                        lambda m, l=l: col(f"b2{l}", m), y_dst, nk=FT)
                x2 = xp.tile([P, DT * T], F32R, tag="xarr")
                if l < L - 1:
                    xb = xbp.tile([P, DT * T], BF16, tag="xbf", name=f"xb_{l + 1}")
                else:
                    xb = None
                layernorm([x2p[:, m * T : (m + 1) * T] for m in range(DT)],
                          f"g2{l}", f"be2{l}", x2, dst_bf=xb)
                x = x2

            # ============================================================ head
            xf = xp.tile([P, DT * T], F32R, tag="xarr")
            layernorm([x[:, m * T : (m + 1) * T] for m in range(DT)], "gf", "bef", xf)
            pyf = psx.tile([1, T], F32, tag="aux")
            for m in range(DT):
                mm(pyf[:], wout_sb[:, m : m + 1], xf[:, m * T : (m + 1) * T],
                   start=(m == 0), stop=(m == DT - 1))
            ysb = sp.tile([1, T], F32, tag="stat1")
            nc.vector.tensor_scalar_add(ysb[:], pyf[:], col("bout", 0, rows=1))
            nc.sync.dma_start(y_out[:], ysb[:])

    nc.compile()
    return nc


# ---------------------------------------------------------------- runner
_SHARED_NAMES = frozenset(
    ["bcols", "WmT", "WaT", "Wq_g", "Wk_g", "WvT", "Wo_g", "W1_g", "W2_g",
     "WoutT", "onesb", "selb"]
)


def _make_runner(nc):
    import jax
    import jax.numpy as jnp
    import concourse.mybir as mybir
    from concourse import bass2jax
    from jax.sharding import Mesh, PartitionSpec
    from jax.experimental.shard_map import shard_map

    bass2jax.install_neuronx_cc_hook()

    partition_name = nc.partition_id_tensor.name if nc.partition_id_tensor else None
    in_names, out_names, out_avals = [], [], []
    for alloc in nc.m.functions[0].allocations:
        if not isinstance(alloc, mybir.MemoryLocationSet):
            continue
        name = alloc.memorylocations[0].name
        if alloc.kind == "ExternalInput":
            if name != partition_name:
                in_names.append(name)
        elif alloc.kind == "ExternalOutput":
            out_names.append(name)
            out_avals.append(
                jax.core.ShapedArray(tuple(alloc.tensor_shape), mybir.dt.np(alloc.dtype))
            )
    n_params = len(in_names)
    all_in = in_names + out_names + ([partition_name] if partition_name else [])

    def _body(*args):
        operands = list(args)
        if partition_name is not None:
            operands.append(bass2jax.partition_id_tensor())
        return tuple(
            bass2jax._bass_exec_p.bind(
                *operands,
                out_avals=tuple(out_avals),
                in_names=tuple(all_in),
                out_names=tuple(out_names),
                lowering_input_output_aliases=(),
                sim_require_finite=True,
                sim_require_nnan=True,
                nc=nc,
            )
        )

    from jax.sharding import NamedSharding

    devices = jax.devices()[:NCORES]
    mesh = Mesh(np.asarray(devices), ("core",))
    repl_sharding = NamedSharding(mesh, PartitionSpec(None))
    core_sharding = NamedSharding(mesh, PartitionSpec("core"))
    in_specs = tuple(
        PartitionSpec(None) if name in _SHARED_NAMES else PartitionSpec("core")
        for name in in_names
    ) + (PartitionSpec("core"),) * len(out_avals)
    out_specs = (PartitionSpec("core"),) * len(out_avals)
    sharded = jax.jit(
        shard_map(_body, mesh=mesh, in_specs=in_specs, out_specs=out_specs,
                  check_rep=False),
        keep_unused=True,
    )

    class Runner:
        def upload(self, shared, per_core):
            ins = []
            for name in in_names:
                if name in _SHARED_NAMES:
                    ins.append((np.asarray(shared[name]), repl_sharding))
                else:
                    ins.append(
                        (
                            np.concatenate(
                                [np.asarray(per_core[c][name])
                                 for c in range(NCORES)],
                                axis=0,
                            ),
                            core_sharding,
                        )
                    )
            # zero output buffers: uploaded once, reused every dispatch
            # (no donation, so they are never consumed)
            for av in out_avals:
                z = np.zeros((NCORES * av.shape[0], *av.shape[1:]), av.dtype)
                ins.append((z, core_sharding))
            self.in_dev = [jax.device_put(a, s) for a, s in ins]
            jax.block_until_ready(self.in_dev)

        def dispatch(self):
            return sharded(*self.in_dev)

        def collect(self, out_arrs):
            return [
                {
                    name: np.asarray(out_arrs[i]).reshape(NCORES, *out_avals[i].shape)[c]
                    for i, name in enumerate(out_names)
                }
                for c in range(NCORES)
            ]

        def run(self):
            import jax as _jax

            out_arrs = self.dispatch()
            _jax.block_until_ready(out_arrs)
            return self.collect(out_arrs)

    return Runner()


def get_runner(inputs):
    """Build (once) and return the runner with inputs uploaded."""
    global _RUNNER
    shared, per_core, idx = _prep_host(inputs)
    if _RUNNER is None:
        nc = _build(idx)
        _RUNNER = _make_runner(nc)
    _RUNNER.upload(shared, per_core)
    return _RUNNER


def kernel(**inputs) -> np.ndarray:
    runner = get_runner(inputs)
    res = runner.run()
    out = np.zeros((B, S, OUT), np.float32)
    for c in range(NCORES):
        b, chunk = c // GPC, c % GPC
        out[b, chunk * T : (chunk + 1) * T, 0] = res[c]["y"][0]
    return out


# revision 45
# speedup vs baseline: 2.8219x; 1.0513x over previous
"""AstroEconomicTransformer on 8 Trainium2 NeuronCores.

Sharding: 8-way sequence-parallel over the B*S = 2048 tokens (256 tokens
per core; cores 0-3 hold batch 0, cores 4-7 batch 1). Activations live
feature-major on chip (x^T: features on partitions, tokens on the free
dim), so every linear layer is a W^T-stationary matmul with the token
dim streaming. Attention needs the full sequence of K/V per batch
element: each layer runs TWO bf16 AllGathers (K right after the K
projection, V after the V projection) within each group of 4 cores, so
the V projection, Q projection and the first score matmuls overlap the
collectives; everything else (LayerNorm, FFN, projections, softmax) is
token-local.

Attention per head-pair: scores^T = k^T.T @ q^T (keys on partitions,
queries free), exp on the scalar engine with the 1/8 scale folded into
the q eviction and pbias as the activation bias (no max subtraction:
scores are O(1) by construction). Token-major V is laid out per head as
[v 64 | one] so the 65-column ctx matmul emits the softmax denominator
at psum partition 64 of the same accumulation — no separate denominator
matmuls. Both heads of a pair accumulate into one psum bank (h0 in
columns 0:256, h1 in 256:512), so one (1,512) reciprocal covers both
denominators; normalization broadcasts the reciprocals with a K=1
matmul. LayerNorm's rsqrt runs as exp(-0.5*ln(var+eps)) so the Act
engine never leaves the exp/ln table. The V bias is folded on host into
the attention output bias (bo_eff = bo + Wo @ bv, exact because softmax
rows sum to one).
"""

import numpy as np

B, S = 2, 1024
D, H, L, DFF = 1024, 16, 6, 4096
NM, NA, OUT = 10, 20, 1
HD = D // H
EPS = 1e-5

NCORES = 8
GPC = 4  # cores per batch group
T = (B * S) // NCORES  # 256 tokens per core
GROUPS = [[0, 1, 2, 3], [4, 5, 6, 7]]
P = 128
DT = D // P  # 8 feature tiles
FT = DFF // P  # 32 dff tiles
TT = T // P  # 2 token tiles per core
ST = S // P  # 8 key tiles per sequence
NPAIR = H // 2

PV = HD + 1  # 65: per-head v row [v 64 | one]
VROW = H * PV  # 1040
KELEM = D * T  # 262144
VELEM = T * VROW  # 266240

_RUNNER = None
REPS = 1


class _Cols:
    """Allocates columns in the (128, n) bias/constant matrix."""

    def __init__(self):
        self.cols = []

    def add(self, mat):  # mat: (128, n) -> first col index
        i = len(self.cols)
        self.cols.extend(np.asarray(mat, np.float32).T)
        return i

    def array(self):
        return np.stack(self.cols, axis=1).astype(np.float32)


def _group_kxm(wT, km, mm_):
    """(N, K, M) pre-transposed weight -> (N*mm_, P, km*P) SBUF tile images."""
    n = wT.shape[0]
    g = wT.reshape(n, km, P, mm_, P).transpose(0, 3, 2, 1, 4)
    return np.ascontiguousarray(g.reshape(n * mm_, P, km * P))


def _make_selb():
    s = np.zeros((P, 4 * P), np.float32)
    for j in range(4):
        s[32 * j, j * P : (j + 1) * P] = 1.0
    return s


def _prep_host(inputs):
    f32 = np.float32
    g = {k: np.asarray(v, f32) for k, v in inputs.items()}

    cols = _Cols()
    idx = {}
    bemb = np.concatenate([g["bm"], g["ba"]])
    idx["bemb"] = cols.add(bemb.reshape(DT, P).T)
    for l in range(L):
        idx[f"bq{l}"] = cols.add(g["bq"][l].reshape(DT, P).T)
        idx[f"bk{l}"] = cols.add(g["bk"][l].reshape(DT, P).T)
        bo_eff = g["bo"][l] + g["Wo"][l] @ g["bv"][l]
        idx[f"bo{l}"] = cols.add(bo_eff.reshape(DT, P).T)
        idx[f"b1{l}"] = cols.add(g["b1"][l].reshape(FT, P).T)
        idx[f"b2{l}"] = cols.add(g["b2"][l].reshape(DT, P).T)
        idx[f"g1{l}"] = cols.add(g["ln1_g"][l].reshape(DT, P).T)
        idx[f"be1{l}"] = cols.add(g["ln1_b"][l].reshape(DT, P).T)
        idx[f"g2{l}"] = cols.add(g["ln2_g"][l].reshape(DT, P).T)
        idx[f"be2{l}"] = cols.add(g["ln2_b"][l].reshape(DT, P).T)
        idx[f"pb{l}"] = cols.add(np.tile(g["pbias"][l][None, :], (P, 1)))
    idx["gf"] = cols.add(g["lnf_g"].reshape(DT, P).T)
    idx["bef"] = cols.add(g["lnf_b"].reshape(DT, P).T)
    idx["bout"] = cols.add(np.full((P, 1), g["bout"][0], f32))
    idx["eps"] = cols.add(np.full((P, 1), EPS, f32))
    idx["zero"] = cols.add(np.zeros((P, 1), f32))
    bcols = cols.array()
    idx["_nbc"] = bcols.shape[1]

    import ml_dtypes

    bf = lambda a: np.ascontiguousarray(a).astype(ml_dtypes.bfloat16)
    tr = lambda w: w.transpose(0, 2, 1)
    shared = {
        "bcols": bcols,
        "WmT": np.ascontiguousarray(g["Wm"].T),
        "WaT": np.ascontiguousarray(g["Wa"].T),
        "Wq_g": bf(_group_kxm(tr(g["Wq"]), DT, DT)),
        "Wk_g": bf(_group_kxm(tr(g["Wk"]), DT, DT)),
        "WvT": bf(tr(g["Wv"])),  # rhs-moving, row slabs
        "Wo_g": bf(_group_kxm(tr(g["Wo"]), DT, DT)),
        "W1_g": bf(_group_kxm(tr(g["W1"]), DT, FT)),
        "W2_g": bf(_group_kxm(tr(g["W2"]), FT, DT)),
        "WoutT": np.ascontiguousarray(g["Wout"].T),
        "onesb": np.ones((P, P), f32),
        # selb[:, j*128:(j+1)*128].T @ rec broadcasts rec row 32*j to all
        # 128 partitions (dens live at 32-aligned partitions)
        "selb": _make_selb(),
    }

    per_core = []
    peT_full = np.ascontiguousarray(g["pe"][0].T)
    for c in range(NCORES):
        b, chunk = c // GPC, c % GPC
        r0 = chunk * T
        per_core.append(
            {
                "mktT": np.ascontiguousarray(g["market_data"][b, r0 : r0 + T, :].T),
                "astT": np.ascontiguousarray(g["astro_data"][b, r0 : r0 + T, :].T),
                "peT": np.ascontiguousarray(peT_full[:, r0 : r0 + T]),
            }
        )
    return shared, per_core, idx


# ---------------------------------------------------------------- device kernel
def _build(idx):
    from contextlib import ExitStack

    import concourse.mybir as mybir
    import concourse.tile as tile
    from concourse import bacc

    dt = mybir.dt
    F32, F32R, BF16 = dt.float32, dt.float32r, dt.bfloat16
    FP8 = dt.float8e4
    AF = mybir.ActivationFunctionType
    ALU = mybir.AluOpType

    nc = bacc.Bacc("TRN2", debug=False, num_devices=NCORES)

    NBC = idx["_nbc"]

    mktT = nc.declare_dram_parameter("mktT", [NM, T], F32R, isOutput=False)
    astT = nc.declare_dram_parameter("astT", [NA, T], F32R, isOutput=False)
    peT = nc.declare_dram_parameter("peT", [D, T], F32, isOutput=False)
    bcols_d = nc.declare_dram_parameter("bcols", [P, NBC], F32, isOutput=False)
    WmT = nc.declare_dram_parameter("WmT", [NM, D // 2], F32R, isOutput=False)
    WaT = nc.declare_dram_parameter("WaT", [NA, D // 2], F32R, isOutput=False)
    Wq_g = nc.declare_dram_parameter("Wq_g", [L * DT, P, DT * P], BF16, isOutput=False)
    Wk_g = nc.declare_dram_parameter("Wk_g", [L * DT, P, DT * P], BF16, isOutput=False)
    WvT = nc.declare_dram_parameter("WvT", [L, D, D], BF16, isOutput=False)
    Wo_g = nc.declare_dram_parameter("Wo_g", [L * DT, P, DT * P], BF16, isOutput=False)
    W1_g = nc.declare_dram_parameter("W1_g", [L * FT, P, DT * P], BF16, isOutput=False)
    W2_g = nc.declare_dram_parameter("W2_g", [L * DT, P, FT * P], BF16, isOutput=False)
    WoutT = nc.declare_dram_parameter("WoutT", [D, OUT], F32R, isOutput=False)
    ones_d = nc.declare_dram_parameter("onesb", [P, P], F32R, isOutput=False)
    sel_d = nc.declare_dram_parameter("selb", [P, 4 * P], F32R, isOutput=False)
    y_out = nc.declare_dram_parameter("y", [1, T], F32, isOutput=True)

    KH = KELEM // 2
    k_inA = [nc.dram_tensor(f"k_inA{l}", [KH], FP8) for l in range(L)]
    k_agA = [nc.dram_tensor(f"k_agA{l}", [GPC, KH], FP8) for l in range(L)]
    k_inB = [nc.dram_tensor(f"k_inB{l}", [KH], FP8) for l in range(L)]
    k_agB = [nc.dram_tensor(f"k_agB{l}", [GPC, KH], FP8) for l in range(L)]
    v_in = [nc.dram_tensor(f"v_in{l}", [VELEM], FP8) for l in range(L)]
    v_ag = [nc.dram_tensor(f"v_ag{l}", [GPC, VELEM], FP8) for l in range(L)]
    warm_in = nc.dram_tensor("warm_in", [256], BF16)
    warm_ag = nc.dram_tensor("warm_ag", [GPC, 256], BF16)

    with tile.TileContext(nc) as tc, ExitStack() as ctx:
        def pool(name, bufs, space="SBUF"):
            return ctx.enter_context(tc.tile_pool(name=name, bufs=bufs, space=space))

        singles = pool("singles", 1)
        xp = pool("xarr", 3)
        xbp = pool("xbf", 2)
        qp = pool("qarr", 1)
        cxp = pool("ctxarr", 1)
        hp = pool("harr", 1)
        wp = pool("wrow", 12)  # qkvo/W1 k-groups + WvT row slabs (bf16)
        w2p = pool("w2grp", 4)  # (128,4096) bf16
        kvp = pool("kvloc", 2)
        kp1 = pool("kloc1", 1)
        kvap = pool("kvall", 1)
        exp_p = pool("exparr", 6)
        sqp = pool("sqp", 2)
        bcp = pool("bcp", 1)
        lntp = pool("lntp", 4)
        sp = pool("small", 3)
        embp = pool("embp", 2)
        nrmp = pool("nrmp", 2)
        nrm1 = pool("nrm1", 1)

        ps = pool("ps", 5, space="PSUM")
        psx = pool("psx", 3, space="PSUM")

        # warm up the collective path while the embed inputs stream in
        nc.gpsimd.collective_compute(
            "AllGather", ALU.bypass, replica_groups=GROUPS,
            ins=[warm_in[:].opt()], outs=[warm_ag[:].opt()],
        )
        bc = singles.tile([P, NBC], F32)
        nc.sync.dma_start(bc[:], bcols_d[:])
        onesb = singles.tile([P, P], F32R)
        nc.sync.dma_start(onesb[:], ones_d[:])
        selb = singles.tile([P, 4 * P], F32R)
        nc.sync.dma_start(selb[:], sel_d[:])
        pe_sb = singles.tile([P, DT * T], F32)
        wout_sb = singles.tile([P, DT], F32R)

        def col(name, j=0, rows=P):
            return bc[0:rows, idx[name] + j : idx[name] + j + 1]

        def mm(out, lhsT, rhs, start, stop):
            nc.tensor.matmul(out, lhsT, rhs, start=start, stop=stop)

        for _rep in range(REPS):
            # ======================================================== embed
            x = xp.tile([P, DT * T], F32R, tag="xarr")
            xb = xbp.tile([P, DT * T], BF16, tag="xbf", name="xb_emb")
            in_sb = embp.tile([NA, 2 * T], F32R, tag="embin")
            nc.sync.dma_start(in_sb[0:NM, 0:T], mktT[:])
            nc.sync.dma_start(in_sb[0:NA, T : 2 * T], astT[:])
            wemb = embp.tile([NA, D // 2], F32R, tag="wemb")
            nc.sync.dma_start(wemb[0:NM, :], WmT[:])
            wemb2 = embp.tile([NA, D // 2], F32R, tag="wemb")
            nc.sync.dma_start(wemb2[:], WaT[:])
            if _rep == 0:
                nc.sync.dma_start(pe_sb[:].rearrange("p (a t) -> p a t", t=T),
                                  peT[:].rearrange("(a p) t -> p a t", p=P))
                nc.sync.dma_start(wout_sb[:].rearrange("p (a o) -> p a o", o=OUT),
                                  WoutT[:].rearrange("(a p) o -> p a o", p=P))
            for m in range(DT):
                pm = ps.tile([P, T], F32, tag="ps")
                if m < 4:
                    w, nin, toff, mo = wemb, NM, 0, m
                else:
                    w, nin, toff, mo = wemb2, NA, T, m - 4
                mm(pm[:], w[0:nin, mo * P : (mo + 1) * P], in_sb[0:nin, toff : toff + T],
                   start=True, stop=True)
                nc.vector.scalar_tensor_tensor(
                    x[:, m * T : (m + 1) * T], pm[:], col("bemb", m),
                    pe_sb[:, m * T : (m + 1) * T], ALU.add, ALU.add,
                )
                nc.vector.tensor_scalar_mul(
                    xb[:, m * T : (m + 1) * T], x[:, m * T : (m + 1) * T], 1.0
                )

            # ============================================================ helpers
            def load_group(pl, src_row, ntiles, tag):
                t = pl.tile([P, ntiles * P], BF16, tag=tag)
                nc.sync.dma_start(t[:], src_row)
                return t

            def proj_fm(wg_d, row0, src, bias_fn, dst_fn, nm=DT, nk=DT, after_m=None):
                """Feature-major projection: dst[m] = act(W @ src + bias)."""
                for m in range(nm):
                    grp = load_group(w2p if nk == FT else wp, wg_d[row0 + m, :, :], nk,
                                     "w2grp" if nk == FT else "wrow")
                    pr = ps.tile([P, T], F32, tag="ps")
                    for kk in range(nk):
                        mm(pr[:], grp[:, kk * P : (kk + 1) * P],
                           src(kk), start=(kk == 0), stop=(kk == nk - 1))
                    dst_fn(m, pr, bias_fn(m))
                    if after_m and m in after_m:
                        after_m[m]()

            def layernorm(src_tiles, gname, bname, dst, dst_bf=None):
                """dst = LN(src) with gamma/beta; Act engine: only Ln+Exp."""
                s_ps = psx.tile([1, T], F32, tag="aux")
                s2_ps = psx.tile([1, T], F32, tag="aux")
                for m in range(DT):
                    eng = nc.gpsimd if m % 2 else nc.vector
                    sq = sqp.tile([P, T], F32R, tag="sq")
                    eng.tensor_mul(sq[:], src_tiles[m], src_tiles[m])
                    mm(s2_ps[:], onesb[:, 0:1], sq[:], start=(m == 0), stop=(m == DT - 1))
                for m in range(DT):
                    mm(s_ps[:], onesb[:, 0:1], src_tiles[m], start=(m == 0), stop=(m == DT - 1))
                mu = sp.tile([1, T], F32R, tag="stat1")
                nc.vector.tensor_scalar_mul(mu[:], s_ps[:], 1.0 / D)
                ex2 = sp.tile([1, T], F32, tag="stat1")
                nc.vector.tensor_scalar_mul(ex2[:], s2_ps[:], 1.0 / D)
                mu2 = sp.tile([1, T], F32, tag="stat1")
                nc.vector.tensor_mul(mu2[:], mu[:], mu[:])
                var = sp.tile([1, T], F32, tag="stat1")
                nc.vector.scalar_tensor_tensor(var[:], mu2[:], -1.0, ex2[:],
                                               ALU.mult, ALU.add)
                # rs = rsqrt(var+eps) = exp(-0.5*ln(var+eps)): stays in the
                # exp/ln act table — no act-func-set reloads anywhere.
                lv = sp.tile([1, T], F32, tag="stat1")
                nc.scalar.activation(lv[:], var[:], AF.Ln,
                                     bias=col("eps", rows=1), scale=1.0)
                rs = sp.tile([1, T], F32R, tag="stat1")
                with nc.allow_low_precision(reason="fp32r feeds the broadcast matmul"):
                    nc.scalar.activation(rs[:], lv[:], AF.Exp,
                                         bias=col("zero", rows=1), scale=-0.5)
                mub_ps = psx.tile([P, T], F32, tag="aux")
                mm(mub_ps[:], onesb[0:1, :], mu[:], start=True, stop=True)
                rsb_ps = psx.tile([P, T], F32, tag="aux")
                mm(rsb_ps[:], onesb[0:1, :], rs[:], start=True, stop=True)
                rsb = bcp.tile([P, T], F32, tag="bcast")
                nc.scalar.copy(rsb[:], rsb_ps[:])
                for m in range(DT):
                    eng = nc.vector if m % 2 else nc.gpsimd
                    eng2 = nc.gpsimd if m % 2 else nc.vector
                    t1 = lntp.tile([P, T], F32, tag="lnt")
                    nc.vector.tensor_sub(t1[:], src_tiles[m], mub_ps[:])
                    t2 = lntp.tile([P, T], F32, tag="lnt")
                    eng.tensor_mul(t2[:], t1[:], rsb[:])
                    eng.tensor_scalar(
                        dst[:, m * T : (m + 1) * T], t2[:], col(gname, m), col(bname, m),
                        ALU.mult, ALU.add,
                    )
                    if dst_bf is not None:
                        eng2.tensor_scalar(
                            dst_bf[:, m * T : (m + 1) * T], t2[:],
                            col(gname, m), col(bname, m), ALU.mult, ALU.add,
                        )

            # ============================================================ layers
            for l in range(L):
                # ---- k projection (feature-major) -> fp8; AG(K) in 2 halves
                karr = kp1.tile([P, DT * T], FP8, tag="kloc", name=f"karr{l}")

                HT = 4 * T

                def k_dst(m, pr, bias, karr=karr, l=l):
                    nc.vector.tensor_scalar_add(karr[:, m * T : (m + 1) * T], pr[:], bias)

                def ag_ka(l=l, karr=karr):
                    # partition-major DRAM image: one contiguous 1KB/partition DMA
                    nc.sync.dma_start(
                        k_inA[l][:].rearrange("(p x) -> p x", p=P), karr[:, 0:HT]
                    )
                    nc.gpsimd.collective_compute(
                        "AllGather", ALU.bypass, replica_groups=GROUPS,
                        ins=[k_inA[l][:].opt()], outs=[k_agA[l][:].opt()],
                    )

                proj_fm(Wk_g, l * DT, lambda kk, xb=xb: xb[:, kk * T : (kk + 1) * T],
                        lambda m, l=l: col(f"bk{l}", m), k_dst, after_m={3: ag_ka})
                nc.sync.dma_start(
                    k_inB[l][:].rearrange("(p x) -> p x", p=P), karr[:, HT : 2 * HT]
                )
                nc.gpsimd.collective_compute(
                    "AllGather", ALU.bypass, replica_groups=GROUPS,
                    ins=[k_inB[l][:].opt()], outs=[k_agB[l][:].opt()],
                )

                # ---- v projection (token-major, per-head [v 64 | one] rows)
                vls = []
                for mt in range(TT):
                    vl = kvp.tile([P, VROW], FP8, tag="vloc")
                    # ones column at [h*65 + 64]
                    nc.vector.memset(
                        vl[:].rearrange("p (h c) -> p h c", c=PV)[:, :, HD:PV], 1.0,
                    )
                    vls.append(vl)
                for n in range(2):
                    pvs = [ps.tile([P, 512], F32, tag="ps", name=f"pv{l}_{n}_{mt}")
                           for mt in range(TT)]
                    for kk in range(DT):
                        slab = wp.tile([P, 512], BF16, tag="wrow")
                        nc.sync.dma_start(
                            slab[:], WvT[l, kk * P : (kk + 1) * P, n * 512 : (n + 1) * 512]
                        )
                        for mt in range(TT):
                            mm(pvs[mt][:], xb[:, kk * T + mt * P : kk * T + (mt + 1) * P],
                               slab[:], start=(kk == 0), stop=(kk == DT - 1))
                    for mt in range(TT):
                        # psum heads 8n..8n+7 -> per-head 65-col slots
                        vl8 = vls[mt][:].rearrange("p (h c) -> p h c", c=PV)[
                            :, 8 * n : 8 * (n + 1), 0:HD
                        ]
                        nc.scalar.copy(
                            vl8, pvs[mt][:].rearrange("p (h d) -> p h d", d=HD),
                        )
                for mt in range(TT):
                    nc.sync.dma_start(
                        v_in[l][:].rearrange("(p x) -> p x", p=P)[
                            :, mt * VROW : (mt + 1) * VROW],
                        vls[mt][:],
                    )
                nc.gpsimd.collective_compute(
                    "AllGather", ALU.bypass, replica_groups=GROUPS,
                    ins=[v_in[l][:].opt()], outs=[v_ag[l][:].opt()],
                )

                # ---- q projection (feature-major; the 1/8 scale is folded into
                # the exp activation's scale); overlaps AGs
                q = qp.tile([P, DT * T], FP8, tag="qarr")

                def q_dst(m, pr, bias, q=q):
                    nc.vector.tensor_scalar_add(q[:, m * T : (m + 1) * T], pr[:], bias)

                proj_fm(Wq_g, l * DT, lambda kk, xb=xb: xb[:, kk * T : (kk + 1) * T],
                        lambda m, l=l: col(f"bq{l}", m), q_dst)

                # ---- bulk-load gathered K/V into SBUF (A halves first: pairs
                # 0-3 score against tiles 0-3 only)
                k_all = kvap.tile([P, GPC * DT * T], FP8, tag="kall", name=f"kall{l}")
                v_all = kvap.tile([P, GPC * TT * VROW], FP8, tag="vall", name=f"vall{l}")
                for c in range(GPC):
                    nc.sync.dma_start(
                        k_all[:, c * DT * T : c * DT * T + HT],
                        k_agA[l][c, :].rearrange("(p x) -> p x", p=P),
                    )
                for c in range(GPC):
                    nc.sync.dma_start(
                        k_all[:, c * DT * T + HT : (c + 1) * DT * T],
                        k_agB[l][c, :].rearrange("(p x) -> p x", p=P),
                    )
                for c in range(GPC):
                    nc.gpsimd.dma_start(
                        v_all[:, c * TT * VROW : (c + 1) * TT * VROW],
                        v_ag[l][c, :].rearrange("(p x) -> p x", p=P),
                    )

                # ---- attention, one head pair per q partition tile.
                # Emit order S(0) S(1) C(0) S(2) C(1) ... so the tensor queue
                # never blocks on exp more than one pair ahead.
                ctxa = cxp.tile([P, DT * T], BF16, tag="ctxarr")
                ctxu = cxp.tile([P, DT * T], F32, tag="ctxu")
                stashes = []
                for g in range(2):
                    st = nrm1.tile([P, 2 * T], F32, tag=f"den{g}", name=f"stash{l}_{g}")
                    # unwritten lanes must stay finite through the reciprocal
                    nc.vector.memset(st[:], 1.0)
                    stashes.append(st)
                eas = [None] * NPAIR

                def scores_phase(p):
                    ea = exp_p.tile([P, ST * T], FP8, tag="exparr")
                    eb = exp_p.tile([P, ST * T], FP8, tag="exparr")
                    eas[p] = (ea, eb)
                    for c in range(GPC):
                        for h01 in range(2):
                            o = h01 * HD
                            pscore = ps.tile([P, 2 * T], F32, tag="ps")
                            for sub in range(TT):
                                kcol = c * DT * T + p * T + sub * P
                                mm(pscore[:, sub * T : (sub + 1) * T],
                                   k_all[o : o + HD, kcol : kcol + P],
                                   q[o : o + HD, p * T : (p + 1) * T], start=True, stop=True)
                            dst = ea if h01 == 0 else eb
                            nc.scalar.activation(
                                dst[:, TT * c * T : TT * (c + 1) * T], pscore[:], AF.Exp,
                                bias=col(f"pb{l}", 2 * p + h01), scale=0.125,
                            )

                def ctx_phase(p):
                    ea, eb = eas[p]
                    # two psum banks (interleaved accumulation chains must not
                    # share a bank): h0 -> A, h1 -> B, dens at partition 64
                    pca = psx.tile([P, T], F32, tag="aux")
                    pcb = psx.tile([P, T], F32, tag="aux")
                    for tk in range(ST):
                        c, sub = tk // TT, tk % TT
                        vcol = c * TT * VROW + sub * VROW
                        mm(pca[0:PV, :],
                           v_all[:, vcol + 2 * p * PV : vcol + (2 * p + 1) * PV],
                           ea[:, tk * T : (tk + 1) * T],
                           start=(tk == 0), stop=(tk == ST - 1))
                        mm(pcb[0:PV, :],
                           v_all[:, vcol + (2 * p + 1) * PV : vcol + (2 * p + 2) * PV],
                           eb[:, tk * T : (tk + 1) * T],
                           start=(tk == 0), stop=(tk == ST - 1))
                    st, j = stashes[p // 4], 32 * (p % 4)
                    nc.vector.tensor_scalar_mul(st[j : j + 1, 0:T],
                                                pca[HD : HD + 1, :], 1.0)
                    nc.vector.tensor_scalar_mul(st[j : j + 1, T : 2 * T],
                                                pcb[HD : HD + 1, :], 1.0)
                    nc.vector.tensor_scalar_mul(ctxu[0:HD, p * T : (p + 1) * T],
                                                pca[0:HD, :], 1.0)
                    nc.vector.tensor_scalar_mul(ctxu[HD : 2 * HD, p * T : (p + 1) * T],
                                                pcb[0:HD, :], 1.0)

                def norm_phase(grp):
                    # one reciprocal covers 4 pairs' denominators (rows 32j)
                    rec4 = nrm1.tile([P, 2 * T], F32R, tag=f"rec{grp}")
                    with nc.allow_low_precision(reason="fp32r feeds the broadcast matmul"):
                        nc.vector.reciprocal(rec4[:], stashes[grp][:])
                    for j in range(4):
                        p = 4 * grp + j
                        pbc = ps.tile([P, 2 * T], F32, tag="ps")
                        mm(pbc[:], selb[:, j * P : (j + 1) * P], rec4[:],
                           start=True, stop=True)
                        bcsb = nrmp.tile([P, 2 * T], F32, tag="bc128")
                        nc.vector.tensor_scalar_mul(bcsb[:], pbc[:], 1.0)
                        nc.gpsimd.tensor_mul(ctxa[0:HD, p * T : (p + 1) * T],
                                             ctxu[0:HD, p * T : (p + 1) * T],
                                             bcsb[0:HD, 0:T])
                        nc.gpsimd.tensor_mul(ctxa[HD : 2 * HD, p * T : (p + 1) * T],
                                             ctxu[HD : 2 * HD, p * T : (p + 1) * T],
                                             bcsb[HD : 2 * HD, T : 2 * T])

                scores_phase(0)
                for p in range(1, NPAIR):
                    scores_phase(p)
                    ctx_phase(p - 1)
                    if p == 4:
                        norm_phase(0)
                ctx_phase(NPAIR - 1)
                norm_phase(1)

                # ---- out projection + residual + LN1
                x1p = xp.tile([P, DT * T], F32R, tag="xarr")

                def o_dst(m, pr, bias, x1p=x1p, x=x):
                    nc.vector.scalar_tensor_tensor(
                        x1p[:, m * T : (m + 1) * T], pr[:], bias,
                        x[:, m * T : (m + 1) * T], ALU.add, ALU.add,
                    )

                proj_fm(Wo_g, l * DT, lambda kk: ctxa[:, kk * T : (kk + 1) * T],
                        lambda m, l=l: col(f"bo{l}", m), o_dst)
                x1 = xp.tile([P, DT * T], F32R, tag="xarr")
                x1b = xbp.tile([P, DT * T], BF16, tag="xbf", name=f"x1b_{l}")
                layernorm([x1p[:, m * T : (m + 1) * T] for m in range(DT)],
                          f"g1{l}", f"be1{l}", x1, dst_bf=x1b)

                # ---- FFN
                harr = hp.tile([P, FT * T], BF16, tag="harr")

                def h_dst(mf, pr, bias, harr=harr):
                    # relu(pr + b1) on the Act engine (Relu is in every table
                    # set); frees the vector engine during the FFN phase
                    nc.scalar.activation(
                        harr[:, mf * T : (mf + 1) * T], pr[:], AF.Relu, bias=bias,
                        scale=1.0,
                    )

                proj_fm(W1_g, l * FT, lambda kk, x1b=x1b: x1b[:, kk * T : (kk + 1) * T],
                        lambda mf, l=l: col(f"b1{l}", mf), h_dst, nm=FT)

                x2p = xp.tile([P, DT * T], F32R, tag="xarr")

                def y_dst(m, pr, bias, x2p=x2p, x1=x1):
                    nc.vector.scalar_tensor_tensor(
                        x2p[:, m * T : (m + 1) * T], pr[:], bias,
                        x1[:, m * T : (m + 1) * T], ALU.add, ALU.add,
                    )

                proj_fm(W2_g, l * DT, lambda kf: harr[:, kf * T : (kf + 1) * T],
                        lambda m, l=l: col(f"b2{l}", m), y_dst, nk=FT)
                x2 = xp.tile([P, DT * T], F32R, tag="xarr")
                if l < L - 1:
                    xb = xbp.tile([P, DT * T], BF16, tag="xbf", name=f"xb_{l + 1}")
                else:
                    xb = None
                layernorm([x2p[:, m * T : (m + 1) * T] for m in range(DT)],
                          f"g2{l}", f"be2{l}", x2, dst_bf=xb)
                x = x2

            # ============================================================ head
            xf = xp.tile([P, DT * T], F32R, tag="xarr")
            layernorm([x[:, m * T : (m + 1) * T] for m in range(DT)], "gf", "bef", xf)
            pyf = psx.tile([1, T], F32, tag="aux")
            for m in range(DT):
                mm(pyf[:], wout_sb[:, m : m + 1], xf[:, m * T : (m + 1) * T],
                   start=(m == 0), stop=(m == DT - 1))
            ysb = sp.tile([1, T], F32, tag="stat1")
            nc.vector.tensor_scalar_add(ysb[:], pyf[:], col("bout", 0, rows=1))
            nc.sync.dma_start(y_out[:], ysb[:])

    nc.compile()
    return nc


# ---------------------------------------------------------------- runner
_SHARED_NAMES = frozenset(
    ["bcols", "WmT", "WaT", "Wq_g", "Wk_g", "WvT", "Wo_g", "W1_g", "W2_g",
     "WoutT", "onesb", "selb"]
)


def _make_runner(nc):
    import jax
    import jax.numpy as jnp
    import concourse.mybir as mybir
    from concourse import bass2jax
    from jax.sharding import Mesh, PartitionSpec
    from jax.experimental.shard_map import shard_map

    bass2jax.install_neuronx_cc_hook()

    partition_name = nc.partition_id_tensor.name if nc.partition_id_tensor else None
    in_names, out_names, out_avals = [], [], []
    for alloc in nc.m.functions[0].allocations:
        if not isinstance(alloc, mybir.MemoryLocationSet):
            continue
        name = alloc.memorylocations[0].name
        if alloc.kind == "ExternalInput":
            if name != partition_name:
                in_names.append(name)
        elif alloc.kind == "ExternalOutput":
            out_names.append(name)
            out_avals.append(
                jax.core.ShapedArray(tuple(alloc.tensor_shape), mybir.dt.np(alloc.dtype))
            )
    n_params = len(in_names)
    all_in = in_names + out_names + ([partition_name] if partition_name else [])

    def _body(*args):
        operands = list(args)
        if partition_name is not None:
            operands.append(bass2jax.partition_id_tensor())
        return tuple(
            bass2jax._bass_exec_p.bind(
                *operands,
                out_avals=tuple(out_avals),
                in_names=tuple(all_in),
                out_names=tuple(out_names),
                lowering_input_output_aliases=(),
                sim_require_finite=True,
                sim_require_nnan=True,
                nc=nc,
            )
        )

    from jax.sharding import NamedSharding

    devices = jax.devices()[:NCORES]
    mesh = Mesh(np.asarray(devices), ("core",))
    repl_sharding = NamedSharding(mesh, PartitionSpec(None))
    core_sharding = NamedSharding(mesh, PartitionSpec("core"))
    in_specs = tuple(
        PartitionSpec(None) if name in _SHARED_NAMES else PartitionSpec("core")
        for name in in_names
    ) + (PartitionSpec("core"),) * len(out_avals)
    out_specs = (PartitionSpec("core"),) * len(out_avals)
    sharded = jax.jit(
        shard_map(_body, mesh=mesh, in_specs=in_specs, out_specs=out_specs,
                  check_rep=False),
        keep_unused=True,
    )

    class Runner:
        def upload(self, shared, per_core):
            ins = []
            for name in in_names:
                if name in _SHARED_NAMES:
                    ins.append((np.asarray(shared[name]), repl_sharding))
                else:
                    ins.append(
                        (
                            np.concatenate(
                                [np.asarray(per_core[c][name])
                                 for c in range(NCORES)],
                                axis=0,
                            ),
                            core_sharding,
                        )
                    )
            # zero output buffers: uploaded once, reused every dispatch
            # (no donation, so they are never consumed)
            for av in out_avals:
                z = np.zeros((NCORES * av.shape[0], *av.shape[1:]), av.dtype)
                ins.append((z, core_sharding))
            self.in_dev = [jax.device_put(a, s) for a, s in ins]
            jax.block_until_ready(self.in_dev)

        def dispatch(self):
            return sharded(*self.in_dev)

        def collect(self, out_arrs):
            return [
                {
                    name: np.asarray(out_arrs[i]).reshape(NCORES, *out_avals[i].shape)[c]
                    for i, name in enumerate(out_names)
                }
                for c in range(NCORES)
            ]

        def run(self):
            import jax as _jax

            out_arrs = self.dispatch()
            _jax.block_until_ready(out_arrs)
            return self.collect(out_arrs)

    return Runner()


def get_runner(inputs):
    """Build (once) and return the runner with inputs uploaded."""
    global _RUNNER
    shared, per_core, idx = _prep_host(inputs)
    if _RUNNER is None:
        nc = _build(idx)
        _RUNNER = _make_runner(nc)
    _RUNNER.upload(shared, per_core)
    return _RUNNER


def kernel(**inputs) -> np.ndarray:
    runner = get_runner(inputs)
    res = runner.run()
    out = np.zeros((B, S, OUT), np.float32)
    for c in range(NCORES):
        b, chunk = c // GPC, c % GPC
        out[b, chunk * T : (chunk + 1) * T, 0] = res[c]["y"][0]
    return out
